# revision 1
# baseline (speedup 1.0000x reference)
"""Trainium2 Bass kernel for BilinearScoringFunction.

scores[b] = relu( einsum('bi,hij,bj->bh', head, W_R, tail)
                  + concat(head, tail) @ V_R.T + b_R ) @ u_R

B=4096, D=512, H=256. Sharded over 8 NeuronCores along the hidden dim H
(32 hidden units per core); each core computes partial u_R dot products
over its hidden slice, and the host sums the 8 partial score vectors.

All matmul operands are bf16 (PSUM accumulation fp32): fp32r stationary
loads take ~218ns/128-col and gate the PE at ~263ns/matmul; bf16 loads
(~107ns) hide under the 512-col streaming time (~215ns), so the PE runs
at the streaming roofline. The tail-side multiply+reduce stays fp32
(VectorE TTR reads fp32 PSUM; tail tile kept fp32 in SBUF).

Per core structure:
  phase 2 (dominant): per h: T_h = head @ W_h on TensorE (4 accumulating
    K=128 matmuls, N=512), then one fused VectorE custom-DVE
    TENSOR_TENSOR_REDUCE computes bil[:, h] = rowsum(T_h * tail) straight
    out of PSUM.
  linear term (inserted into the PE stream after h=5): h-major
    lin^T = V_slice @ concat^T as 8 accumulating matmuls per 512-batch
    tile with the tiny V chunks stationary, bias added in PSUM, then
    32x32 VectorE block transposes back to b-major.
  phase 3: per batch tile: bil + lin (VectorE), relu (ScalarE),
    fused dot with u_slice (VectorE custom-DVE reduce).

DMA order: W_0 first, then head^T/tail as interleaved per-batch-tile
slices, so the first matmul quad gates on ~0.8MB and h=0/h=1 run
DMA-paced as the streams land.
"""

import os
from contextlib import ExitStack

import numpy as np
import ml_dtypes

import concourse.bacc as bacc
import concourse.tile as tile
import concourse.mybir as mybir
from concourse import bass_utils
from concourse.dve_ops import TENSOR_TENSOR_REDUCE

B, D, H = 4096, 512, 256
NCORES = 8
HSL = H // NCORES          # hidden units per core = 32
P = 128                    # partitions
BT = B // P                # batch tiles of 128 = 32
NB5 = B // 512             # batch tiles of 512 = 8
KD = D // P                # contraction chunks per operand = 4
KC = 2 * KD                # concat contraction chunks = 8
LIN_AT_H = 6               # insert linear-term matmuls before this h
FP8_HS = (8, 14, 20, 26)   # h's computed in fp8 e4m3 DoubleRow (2x PE rate)

_F32 = mybir.dt.float32
_BF16 = mybir.dt.bfloat16
_F8 = mybir.dt.float8e4

_NC_CACHE = None


def _build_nc():
    nc = bacc.Bacc(
        "TRN2",
        target_bir_lowering=False,
        debug=False,
        enable_asserts=False,
        num_devices=NCORES,
    )
    # all pre-arranged host-side so every DMA is a clean 2D/3D copy
    hT = nc.dram_tensor("hT", [P, KD, B], _BF16, kind="ExternalInput").ap()
    hT8 = nc.dram_tensor("hT8", [P, KD, B], _F8, kind="ExternalInput").ap()
    tT = nc.dram_tensor("tT", [P, KD, B], _BF16, kind="ExternalInput").ap()
    tl = nc.dram_tensor("tl", [B, D], _BF16, kind="ExternalInput").ap()
    w = nc.dram_tensor("w", [HSL, P, KD, D], _BF16, kind="ExternalInput").ap()
    w8 = nc.dram_tensor("w8", [len(FP8_HS), P, KD, D], _F8,
                        kind="ExternalInput").ap()
    vc = nc.dram_tensor("vc", [P, KC, HSL], _BF16, kind="ExternalInput").ap()
    ub = nc.dram_tensor("ub", [P, HSL], _F32, kind="ExternalInput").ap()
    br = nc.dram_tensor("br", [P, 1], _F32, kind="ExternalInput").ap()
    out = nc.dram_tensor("scores_part", [P, BT], _F32, kind="ExternalOutput").ap()

    with tile.TileContext(nc) as tc, ExitStack() as ctx:
        const = ctx.enter_context(tc.tile_pool(name="const", bufs=1))
        wp = ctx.enter_context(tc.tile_pool(name="w", bufs=4))
        psp = ctx.enter_context(tc.tile_pool(name="ps", bufs=8, space="PSUM"))
        scr = ctx.enter_context(tc.tile_pool(name="scr", bufs=2))

        # --- DMAs in priority order: compute start gates on W[0] + hT only.
        w_tiles = {}

        def load_w(h):
            w_t = wp.tile([P, KD, D], _BF16, name="wt")
            nc.sync.dma_start(w_t[:], w[h])
            return w_t

        # w[0] in two halves: the first matmul gates on the k=0,1 half only
        w_tiles[0] = wp.tile([P, KD, D], _BF16, name="wt")
        nc.sync.dma_start(w_tiles[0][:, 0:2, :], w[0][:, 0:2, :])
        nc.sync.dma_start(w_tiles[0][:, 2:4, :], w[0][:, 2:4, :])

        # head^T / tail as interleaved 2-batch-tile chunks: the h=0 matmul
        # quad for tile bt only gates on its 256KB hT chunk, so compute
        # starts early and h=0/h=1 run compute-paced (each dma_start costs
        # ~740ns of Sync-engine issue time, so 1-bt chunks would pace the
        # PE at the trigger rate instead).
        hT_t = const.tile([P, KD, B], _BF16)
        tT_t = const.tile([P, KD, B], _BF16)
        tl_t = const.tile([P, BT, D], _BF16)
        # bt=0 alone first (halves the bytes gating the first matmul), its
        # two slice triggers on the idle Scalar queue so they issue in
        # parallel with w[0]'s trigger on Sync; w[1] right after so
        # quad(1, 0) isn't trigger-queued behind later slices.
        nc.scalar.dma_start(hT_t[:, :, 0:P], hT[:, :, 0:P])
        nc.scalar.dma_start(tl_t[:, 0:1, :], tl[0:P, :].rearrange(
            "(t p) d -> p t d", p=P))
        w_tiles[1] = load_w(1)
        nc.sync.dma_start(hT_t[:, :, P:2 * P], hT[:, :, P:2 * P])
        nc.sync.dma_start(tl_t[:, 1:2, :], tl[P:2 * P, :].rearrange(
            "(t p) d -> p t d", p=P))
        for bt in range(2, BT, 2):
            sl = slice(bt * P, (bt + 2) * P)
            nc.sync.dma_start(hT_t[:, :, sl], hT[:, :, sl])
            nc.sync.dma_start(tl_t[:, bt:bt + 2, :], tl[sl, :].rearrange(
                "(t p) d -> p t d", p=P))
            if bt == 8:
                # w[2]/w[3] here: late enough not to displace the first
                # stream chunks on the wire, early enough to land well
                # before h=2 consumes them (~68us).
                w_tiles[2] = load_w(2)
                w_tiles[3] = load_w(3)
        # tail^T is only needed by the linear phase (inserted at h=5,
        # ~150us in): one bulk DMA, issued after the streams above. Same
        # for the fp8 head copy (first used at h=8) and fp8 W tiles.
        nc.sync.dma_start(tT_t[:], tT[:])
        hT8_t = const.tile([P, KD, B], _F8)
        nc.sync.dma_start(hT8_t[:], hT8[:])
        w8_tiles = []
        for i in range(len(FP8_HS)):
            w8_t = const.tile([P, KD, D], _F8, name=f"w8_{i}")
            nc.sync.dma_start(w8_t[:], w8[i])
            w8_tiles.append(w8_t)

        vc_t = const.tile([P, KC, HSL], _BF16)
        nc.sync.dma_start(vc_t[:], vc[:])
        ub_t = const.tile([P, HSL], _F32)
        nc.sync.dma_start(ub_t[:], ub[:, :])
        br_t = const.tile([P, 1], _F32)
        nc.sync.dma_start(br_t[:], br[:, :])

        bil_t = const.tile([P, BT, HSL], _F32)   # pure bilinear, b-major
        linb_t = const.tile([P, BT, HSL], _F32)  # linear + bias, b-major
        scores_t = const.tile([P, BT], _F32)

        lsp = ctx.enter_context(tc.tile_pool(name="lst", bufs=2))

        def lin_phase():
            # col-tiled: 4 batch-512 tiles accumulate concurrently in the
            # four 32-partition column groups of one PSUM bank. pl[32j+c, n]
            # = lin^T[h=c, b=(rnd*4+j)*512+n]; per-col-group accumulation
            # groups are independent (zero regions are per-partition-range).
            for rnd in range(2):
                # shares the quad pool (same shape/tag -> same slots), so all
                # 8 PSUM banks back the quad stream outside the lin window
                pl = psp.tile([P, 512], _F32, name="ps")
                for kc in range(KC):
                    for j in range(4):
                        b512 = rnd * 4 + j
                        if kc < KD:
                            rhs = hT_t[:, kc, b512 * 512:(b512 + 1) * 512]
                        else:
                            rhs = tT_t[:, kc - KD, b512 * 512:(b512 + 1) * 512]
                        nc.tensor.matmul(
                            pl[32 * j:32 * (j + 1), :], vc_t[:, kc, :], rhs,
                            start=(kc == 0), stop=(kc == KC - 1),
                            tile_position=(0, 32 * j),
                        )
                # bias add in place (per-partition scalar = b_R tiled 4x)
                nc.vector.tensor_scalar_add(pl[:], pl[:], br_t[:])
                # transpose all 64 32x32 blocks in one DVE op
                lin_stage = lsp.tile([P, 512], _F32, name="lst")
                nc.vector.transpose(lin_stage[:], pl[:])
                # scatter to b-major linb_t: element [32j+r, 32(4q+m)+c] is
                # lin[h=c, b=(rnd*4+j)*512+32(4q+m)+r] -> partition 32m+r,
                # bt=(rnd*4+j)*4+q. One SBUF->SBUF DMA per (j, m).
                for j in range(4):
                    blk = lin_stage[32 * j:32 * (j + 1), :].rearrange(
                        "p (q m c) -> p q m c", q=4, m=4
                    )
                    for m in range(4):
                        dst = linb_t[32 * m:32 * (m + 1),
                                     (rnd * 4 + j) * 4:(rnd * 4 + j) * 4 + 4, :]
                        nc.sync.dma_start(dst, blk[:, :, m, :])

        # --- Phase 2: per h: T_h = head @ W_h ; bil[:, h] = rowsum(T_h * tail)
        # On the last h, phase-3 relu prep is interleaved per batch tile.
        s2p = ctx.enter_context(tc.tile_pool(name="s2", bufs=2))

        def _udot(bt):
            # scores_part[b] = relu(bil + lin)[b, :] @ u_slice
            s2_t = s2p.tile([P, HSL], _F32, name="s2")
            nc.vector._custom_dve(
                TENSOR_TENSOR_REDUCE,
                out=s2_t[:],
                in0=bil_t[:, bt, :],
                in1=ub_t[:],
                s0=0.0,
                s1=1.0,
                accum_out=scores_t[:, bt:bt + 1],
            )

        def quad(h, bt, w_t, fp8=False):
            ps_t = psp.tile([P, D], _F32, name="ps")
            if fp8:
                # DoubleRow: K=256 per matmul via [Ki, 2, dim] pair APs
                for c in range(2):
                    nc.tensor.matmul(
                        ps_t[:],
                        hT8_t[:, 2 * c:2 * c + 2, bt * P:(bt + 1) * P],
                        w_t[:, 2 * c:2 * c + 2, :],
                        start=(c == 0),
                        stop=(c == 1),
                        perf_mode=mybir.MatmulPerfMode.DoubleRow,
                    )
            else:
                for k in range(KD):
                    nc.tensor.matmul(
                        ps_t[:],
                        hT_t[:, k, bt * P:(bt + 1) * P],
                        w_t[:, k, :],
                        start=(k == 0),
                        stop=(k == KD - 1),
                    )
            s_t = scr.tile([P, D], _F32, name="s")
            nc.vector._custom_dve(
                TENSOR_TENSOR_REDUCE,
                out=s_t[:],
                in0=ps_t[:],
                in1=tl_t[:, bt, :],
                s0=0.0,
                s1=1.0,
                accum_out=bil_t[:, bt, h:h + 1],
            )

        # h=0 and h=1 interleaved per bt: during this window the hT/tl
        # streams are still landing, so give the PE 2 quads per arriving tile.
        for bt in range(BT):
            quad(0, bt, w_tiles[0])
            quad(1, bt, w_tiles[1])
        w_tiles.pop(0)
        w_tiles.pop(1)

        bf16_seq = [h for h in range(2, HSL) if h not in FP8_HS]
        pre = 2  # bf16_seq[0:2] == (2, 3) already in flight
        start_bt = 0  # quads of this h already pulled into the previous fp8 h
        for h in range(2, HSL):
            fp8 = h in FP8_HS
            if fp8:
                w_t = w8_tiles[FP8_HS.index(h)]
            else:
                if pre < len(bf16_seq):
                    w_tiles[bf16_seq[pre]] = load_w(bf16_seq[pre])
                    pre += 1
                w_t = w_tiles.pop(h)
            if h == HSL - 2:
                # defer: h=30 runs bt-major merged with h=31, so the
                # phase-3 VectorE work (add+udot on top of the TTRs)
                # spreads over two quads of PE time per batch tile
                w30 = w_t
                continue
            first_bt, start_bt = start_bt, 0
            for bt in range(first_bt, BT):
                if h == HSL - 1:
                    quad(HSL - 2, bt, w30)
                quad(h, bt, w_t, fp8=fp8)
                if fp8 and bt % 2 == 1:
                    # software-pipeline across the h boundary: one bf16 quad
                    # of h+1 per 2 fp8 quads. Keeps the VectorE reduce lag
                    # inside the 8-bank PSUM window and gives DR pairs an
                    # adjacent bf16 matmul to hide the 256-col weight load
                    # under (denser 1:1 interleave measured Vector-clamped).
                    quad(h + 1, start_bt, w_tiles[h + 1], fp8=False)
                    start_bt += 1
                if h == HSL - 1:
                    # in-place: bil := relu(bil + lin)
                    nc.vector.tensor_add(
                        bil_t[:, bt, :], bil_t[:, bt, :], linb_t[:, bt, :]
                    )
                    if bt == BT - 1:
                        # final tile: keep the whole tail chain on VectorE
                        # (no ScalarE hop on the critical path)
                        nc.vector.tensor_scalar_max(
                            bil_t[:, bt, :], bil_t[:, bt, :], 0.0
                        )
                    else:
                        nc.scalar.activation(
                            bil_t[:, bt, :], bil_t[:, bt, :],
                            mybir.ActivationFunctionType.Relu,
                        )
                    if bt >= 1:
                        _udot(bt - 1)
                    if bt - 1 == 15:
                        nc.sync.dma_start(out[:, 0:16], scores_t[:, 0:16])
                    if bt - 1 == 30:
                        nc.sync.dma_start(out[:, 16:31], scores_t[:, 16:31])
            if h == LIN_AT_H - 1:
                lin_phase()

        _udot(BT - 1)
        nc.sync.dma_start(out[:, 31:BT], scores_t[:, 31:BT])

    nc.compile()
    return nc


def _get_nc():
    global _NC_CACHE
    if _NC_CACHE is None:
        _NC_CACHE = _build_nc()
    return _NC_CACHE


def kernel(head_embeddings, relation_embeddings, tail_embeddings, W_R, V_R, u_R, b_R):
    head = np.asarray(head_embeddings, dtype=np.float32)
    tail = np.asarray(tail_embeddings, dtype=np.float32)
    W = np.asarray(W_R, dtype=np.float32)
    V = np.asarray(V_R, dtype=np.float32)
    u = np.asarray(u_R, dtype=np.float32)
    b = np.asarray(b_R, dtype=np.float32)

    bf = ml_dtypes.bfloat16
    f8 = ml_dtypes.float8_e4m3fn
    # [D, B] -> [P, KD, B]: partition p holds row k*128+p of the transpose
    hTr = head.T.reshape(KD, P, B).transpose(1, 0, 2)
    hTa = np.ascontiguousarray(hTr.astype(bf))
    hT8a = np.ascontiguousarray(hTr.astype(f8))
    tTa = np.ascontiguousarray(
        tail.T.reshape(KD, P, B).transpose(1, 0, 2).astype(bf))

    in_maps = []
    for c in range(NCORES):
        hs = slice(c * HSL, (c + 1) * HSL)
        # W[hs]: [HSL, D, D] -> [HSL, P, KD, D]
        wr = W[hs].reshape(HSL, KD, P, D).transpose(0, 2, 1, 3)
        wa = np.ascontiguousarray(wr.astype(bf))
        w8a = np.ascontiguousarray(wr[list(FP8_HS)].astype(f8))
        # V[hs].T: [2D, HSL] -> [P, KC, HSL]
        vca = np.ascontiguousarray(
            V[hs].T.reshape(KC, P, HSL).transpose(1, 0, 2).astype(bf))
        in_maps.append({
            "hT": hTa,
            "hT8": hT8a,
            "tT": tTa,
            "tl": tail.astype(bf),
            "w": wa,
            "w8": w8a,
            "vc": vca,
            "ub": np.ascontiguousarray(np.broadcast_to(u[hs], (P, HSL))),
            "br": np.ascontiguousarray(np.tile(b[hs], 4).reshape(P, 1)),
        })

    nc = _get_nc()
    trace = bool(int(os.environ.get("BILINEAR_TRACE", "0")))
    res = bass_utils.run_bass_kernel_spmd(
        nc, in_maps, core_ids=list(range(NCORES)), trace=trace
    )
    global LAST_RESULT
    LAST_RESULT = res
    if trace:
        print(f"HW exec time: {res.exec_time_ns} ns")
        if res.instructions_and_trace:
            print(f"trace: {res.instructions_and_trace[1]}")

    acc = np.zeros(B, dtype=np.float64)
    for c in range(NCORES):
        part = res.results[c]["scores_part"]  # [P, BT]
        acc += part.T.reshape(-1).astype(np.float64)
    return acc.astype(np.float32)



# revision 2
# speedup vs baseline: 1.0661x; 1.0661x over previous
"""Trainium2 Bass kernel for BilinearScoringFunction.

scores[b] = relu( einsum('bi,hij,bj->bh', head, W_R, tail)
                  + concat(head, tail) @ V_R.T + b_R ) @ u_R

B=4096, D=512, H=256. Sharded over 8 NeuronCores along the hidden dim H
(32 hidden units per core); the host sums the 8 partial score vectors.

Precision plan (validated against the reference in fp64 simulation):
  - h's are assigned to cores by |u_h| rank: the 144 smallest-|u| h's run
    as fp8-e4m3 DoubleRow matmuls (2x PE rate; 18 per core), the 112
    largest-|u| h's in bf16 (14 per core). Since each h's score-error
    contribution scales with u_h^2, sorting buys ~4.5x more fp8 h's than
    the baseline random assignment at the same error.
  - a relu mean-correction: relu(x) = x/2 + |x|/2, and the linear half of
    the fp8 quantization error collapses over h:
      sum_h u_h * (bil_h - bil8_h) = head@EWu@tail + eh@W8u@tail
    with EWu = sum u_h (W_h - W8_h) (bf16), eh = head - fp8(head) (fp8),
    W8u = sum u_h W8_h (fp8). Computed as one extra pseudo-h column
    (4 bf16 + 2 DR matmuls into one PSUM + one TTR), weighted +0.5 in the
    u-dot and excluded from the relu. Cuts the fp8 error ~sqrt(2)x.
  Simulated rel_l2 = 1.37e-2 (gate 2e-2).

Per core structure (PE ~690us, Vector ~745us -> Vector-bound):
  per h: T_h = head @ W_h on TensorE (4 accumulating K=128 bf16 matmuls
  or 2 K=256 fp8-DR matmuls, N=512), then one fused VectorE custom-DVE
  TENSOR_TENSOR_REDUCE computes bil[:, h] = rowsum(T_h * tail) straight
  out of PSUM. Linear term via h-major V@concat^T matmuls + 32x32 DVE
  transposes (as in the earlier revision). Phase 3 (add lin, relu on the
  32 real columns, u-dot over 33 columns) interleaved with the last two
  h's, bt-major.
"""

import os
from contextlib import ExitStack

import numpy as np
import ml_dtypes

import concourse.bacc as bacc
import concourse.tile as tile
import concourse.mybir as mybir
from concourse import bass_utils
from concourse.dve_ops import TENSOR_TENSOR_REDUCE

B, D, H = 4096, 512, 256
NCORES = 8
HSL = H // NCORES          # hidden units per core = 32
N8C = 18                   # fp8-DR h's per core (slots 0..N8C-1)
NBF = HSL - N8C            # bf16 h's per core (slots N8C..31)
NCOL = HSL + 1             # bil columns incl. the correction column
P = 128                    # partitions
BT = B // P                # batch tiles of 128 = 32
KD = D // P                # contraction chunks per operand = 4
KC = 2 * KD                # concat contraction chunks = 8
LIN_AT = 3                 # insert linear-term matmuls after this many
                           # h-loop rounds of the main sequence

_F32 = mybir.dt.float32
_BF16 = mybir.dt.bfloat16
_F8 = mybir.dt.float8e4

_NC_CACHE = None


def _build_nc():
    nc = bacc.Bacc(
        "TRN2",
        target_bir_lowering=False,
        debug=False,
        enable_asserts=False,
        num_devices=NCORES,
    )
    # all pre-arranged host-side so every DMA is a clean 2D/3D copy
    hT = nc.dram_tensor("hT", [P, KD, B], _BF16, kind="ExternalInput").ap()
    hT8 = nc.dram_tensor("hT8", [P, KD, B], _F8, kind="ExternalInput").ap()
    ehT8 = nc.dram_tensor("ehT8", [P, KD, B], _F8, kind="ExternalInput").ap()
    tT = nc.dram_tensor("tT", [P, KD, B], _BF16, kind="ExternalInput").ap()
    tl = nc.dram_tensor("tl", [B, D], _BF16, kind="ExternalInput").ap()
    w = nc.dram_tensor("w", [NBF, P, KD, D], _BF16, kind="ExternalInput").ap()
    w8 = nc.dram_tensor("w8", [N8C, P, KD, D], _F8, kind="ExternalInput").ap()
    ewu = nc.dram_tensor("ewu", [P, KD, D], _BF16, kind="ExternalInput").ap()
    w8u8 = nc.dram_tensor("w8u8", [P, KD, D], _F8, kind="ExternalInput").ap()
    vc = nc.dram_tensor("vc", [P, KC, HSL], _BF16, kind="ExternalInput").ap()
    ub = nc.dram_tensor("ub", [P, NCOL], _F32, kind="ExternalInput").ap()
    br = nc.dram_tensor("br", [P, 1], _F32, kind="ExternalInput").ap()
    out = nc.dram_tensor("scores_part", [P, BT], _F32, kind="ExternalOutput").ap()

    with tile.TileContext(nc) as tc, ExitStack() as ctx:
        const = ctx.enter_context(tc.tile_pool(name="const", bufs=1))
        wp = ctx.enter_context(tc.tile_pool(name="w", bufs=4))
        w8p = ctx.enter_context(tc.tile_pool(name="w8", bufs=6))
        psp = ctx.enter_context(tc.tile_pool(name="ps", bufs=8, space="PSUM"))
        scr = ctx.enter_context(tc.tile_pool(name="scr", bufs=2))

        # processing order of per-core h slots: two bf16 h's first (they
        # pace the hT/tl stream landing), then the rest; the last two bf16
        # slots are merged with phase 3.
        first2 = [N8C, N8C + 1]
        mid_seq = list(range(N8C)) + list(range(N8C + 2, HSL - 2))
        last2 = [HSL - 2, HSL - 1]

        def is_f8(h):
            return h < N8C

        w_tiles = {}

        def load_w(h):
            if is_f8(h):
                t = w8p.tile([P, KD, D], _F8, name="w8t")
                nc.sync.dma_start(t[:], w8[h])
            else:
                t = wp.tile([P, KD, D], _BF16, name="wt")
                nc.sync.dma_start(t[:], w[h - N8C])
            w_tiles[h] = t
            return t

        # --- DMAs in priority order: compute start gates on w[first2[0]] + hT.
        w_tiles[first2[0]] = wp.tile([P, KD, D], _BF16, name="wt")
        nc.sync.dma_start(w_tiles[first2[0]][:, 0:2, :], w[first2[0] - N8C][:, 0:2, :])
        nc.sync.dma_start(w_tiles[first2[0]][:, 2:4, :], w[first2[0] - N8C][:, 2:4, :])

        # head^T / tail as interleaved 2-batch-tile chunks: the first-h matmul
        # quad for tile bt only gates on its 256KB hT chunk, so compute
        # starts early and the first two h's run DMA-paced as streams land.
        hT_t = const.tile([P, KD, B], _BF16)
        tT_t = const.tile([P, KD, B], _BF16)
        tl_t = const.tile([P, BT, D], _BF16)
        nc.scalar.dma_start(hT_t[:, :, 0:P], hT[:, :, 0:P])
        nc.scalar.dma_start(tl_t[:, 0:1, :], tl[0:P, :].rearrange(
            "(t p) d -> p t d", p=P))
        load_w(first2[1])
        nc.sync.dma_start(hT_t[:, :, P:2 * P], hT[:, :, P:2 * P])
        nc.sync.dma_start(tl_t[:, 1:2, :], tl[P:2 * P, :].rearrange(
            "(t p) d -> p t d", p=P))
        for bt in range(2, BT, 2):
            sl = slice(bt * P, (bt + 2) * P)
            nc.sync.dma_start(hT_t[:, :, sl], hT[:, :, sl])
            nc.sync.dma_start(tl_t[:, bt:bt + 2, :], tl[sl, :].rearrange(
                "(t p) d -> p t d", p=P))
            if bt == 8:
                # correction + first mid-seq weights: late enough not to
                # displace the stream chunks, early enough to land before use
                ewu_t = const.tile([P, KD, D], _BF16)
                nc.sync.dma_start(ewu_t[:], ewu[:])
                w8u8_t = const.tile([P, KD, D], _F8)
                nc.sync.dma_start(w8u8_t[:], w8u8[:])
                load_w(mid_seq[0])
                load_w(mid_seq[1])
        # bulk tensors needed later: fp8 head copies, tail^T (lin phase)
        hT8_t = const.tile([P, KD, B], _F8)
        nc.sync.dma_start(hT8_t[:], hT8[:])
        ehT8_t = const.tile([P, KD, B], _F8)
        nc.sync.dma_start(ehT8_t[:], ehT8[:])
        nc.sync.dma_start(tT_t[:], tT[:])

        vc_t = const.tile([P, KC, HSL], _BF16)
        nc.sync.dma_start(vc_t[:], vc[:])
        ub_t = const.tile([P, NCOL], _F32)
        nc.sync.dma_start(ub_t[:], ub[:, :])
        br_t = const.tile([P, 1], _F32)
        nc.sync.dma_start(br_t[:], br[:, :])

        bil_t = const.tile([P, BT, NCOL], _F32)  # bilinear + corr col, b-major
        linb_t = const.tile([P, BT, HSL], _F32)  # linear + bias, b-major
        scores_t = const.tile([P, BT], _F32)

        lsp = ctx.enter_context(tc.tile_pool(name="lst", bufs=2))

        def lin_phase():
            # col-tiled: 4 batch-512 tiles accumulate concurrently in the
            # four 32-partition column groups of one PSUM bank. pl[32j+c, n]
            # = lin^T[h=c, b=(rnd*4+j)*512+n].
            for rnd in range(2):
                pl = psp.tile([P, 512], _F32, name="ps")
                for kc in range(KC):
                    for j in range(4):
                        b512 = rnd * 4 + j
                        if kc < KD:
                            rhs = hT_t[:, kc, b512 * 512:(b512 + 1) * 512]
                        else:
                            rhs = tT_t[:, kc - KD, b512 * 512:(b512 + 1) * 512]
                        nc.tensor.matmul(
                            pl[32 * j:32 * (j + 1), :], vc_t[:, kc, :], rhs,
                            start=(kc == 0), stop=(kc == KC - 1),
                            tile_position=(0, 32 * j),
                        )
                nc.vector.tensor_scalar_add(pl[:], pl[:], br_t[:])
                lin_stage = lsp.tile([P, 512], _F32, name="lst")
                nc.vector.transpose(lin_stage[:], pl[:])
                for j in range(4):
                    blk = lin_stage[32 * j:32 * (j + 1), :].rearrange(
                        "p (q m c) -> p q m c", q=4, m=4
                    )
                    for m in range(4):
                        dst = linb_t[32 * m:32 * (m + 1),
                                     (rnd * 4 + j) * 4:(rnd * 4 + j) * 4 + 4, :]
                        nc.sync.dma_start(dst, blk[:, :, m, :])

        s2p = ctx.enter_context(tc.tile_pool(name="s2", bufs=2))

        def _udot(bt):
            # scores_part[b] = (relu(bil + lin) ++ corr)[b, :] @ (u ++ 0.5)
            s2_t = s2p.tile([P, NCOL], _F32, name="s2")
            nc.vector._custom_dve(
                TENSOR_TENSOR_REDUCE,
                out=s2_t[:],
                in0=bil_t[:, bt, :],
                in1=ub_t[:],
                s0=0.0,
                s1=1.0,
                accum_out=scores_t[:, bt:bt + 1],
            )

        def _ttr(ps_t, bt, col):
            s_t = scr.tile([P, D], _F32, name="s")
            nc.vector._custom_dve(
                TENSOR_TENSOR_REDUCE,
                out=s_t[:],
                in0=ps_t[:],
                in1=tl_t[:, bt, :],
                s0=0.0,
                s1=1.0,
                accum_out=bil_t[:, bt, col:col + 1],
            )

        def quad(h, bt, w_t):
            ps_t = psp.tile([P, D], _F32, name="ps")
            if is_f8(h):
                for c in range(2):
                    nc.tensor.matmul(
                        ps_t[:],
                        hT8_t[:, 2 * c:2 * c + 2, bt * P:(bt + 1) * P],
                        w_t[:, 2 * c:2 * c + 2, :],
                        start=(c == 0),
                        stop=(c == 1),
                        perf_mode=mybir.MatmulPerfMode.DoubleRow,
                    )
            else:
                for k in range(KD):
                    nc.tensor.matmul(
                        ps_t[:],
                        hT_t[:, k, bt * P:(bt + 1) * P],
                        w_t[:, k, :],
                        start=(k == 0),
                        stop=(k == KD - 1),
                    )
            _ttr(ps_t, bt, h)

        def corr_quad(bt):
            # bil[:, 32] = head@EWu@tail + eh8@W8u8@tail  (= sum_h u_h * fp8
            # quantization error over the DR h's, udot weight +0.5)
            ps_t = psp.tile([P, D], _F32, name="ps")
            for k in range(KD):
                nc.tensor.matmul(
                    ps_t[:],
                    hT_t[:, k, bt * P:(bt + 1) * P],
                    ewu_t[:, k, :],
                    start=(k == 0), stop=False,
                )
            for c in range(2):
                nc.tensor.matmul(
                    ps_t[:],
                    ehT8_t[:, 2 * c:2 * c + 2, bt * P:(bt + 1) * P],
                    w8u8_t[:, 2 * c:2 * c + 2, :],
                    start=False, stop=(c == 1),
                    perf_mode=mybir.MatmulPerfMode.DoubleRow,
                )
            _ttr(ps_t, bt, HSL)

        # --- main sequence ---
        # first two bf16 h's interleaved per bt while streams land
        for bt in range(BT):
            quad(first2[0], bt, w_tiles[first2[0]])
            quad(first2[1], bt, w_tiles[first2[1]])
        w_tiles.pop(first2[0])
        w_tiles.pop(first2[1])

        # correction column (needs hT fully landed + ewu/w8u8/ehT8)
        for bt in range(BT):
            corr_quad(bt)

        pre = 2  # mid_seq[0:2] already in flight
        for idx, h in enumerate(mid_seq):
            if pre < len(mid_seq):
                load_w(mid_seq[pre])
                pre += 1
            elif pre < len(mid_seq) + 2:
                load_w(last2[pre - len(mid_seq)])
                pre += 1
            w_t = w_tiles.pop(h)
            for bt in range(BT):
                quad(h, bt, w_t)
            if idx == LIN_AT - 1:
                lin_phase()

        # last two h's merged bt-major with phase 3
        h30, h31 = last2
        w30 = w_tiles.pop(h30)
        w31 = w_tiles.pop(h31)
        for bt in range(BT):
            quad(h30, bt, w30)
            quad(h31, bt, w31)
            # in-place: bil := relu(bil + lin) on the 32 real columns
            nc.vector.tensor_add(
                bil_t[:, bt, 0:HSL], bil_t[:, bt, 0:HSL], linb_t[:, bt, :]
            )
            if bt == BT - 1:
                nc.vector.tensor_scalar_max(
                    bil_t[:, bt, 0:HSL], bil_t[:, bt, 0:HSL], 0.0
                )
            else:
                nc.scalar.activation(
                    bil_t[:, bt, 0:HSL], bil_t[:, bt, 0:HSL],
                    mybir.ActivationFunctionType.Relu,
                )
            if bt >= 1:
                _udot(bt - 1)
            if bt - 1 == 15:
                nc.sync.dma_start(out[:, 0:16], scores_t[:, 0:16])
            if bt - 1 == 30:
                nc.sync.dma_start(out[:, 16:31], scores_t[:, 16:31])

        _udot(BT - 1)
        nc.sync.dma_start(out[:, 31:BT], scores_t[:, 31:BT])

    nc.compile()
    return nc


def _get_nc():
    global _NC_CACHE
    if _NC_CACHE is None:
        _NC_CACHE = _build_nc()
    return _NC_CACHE


def kernel(head_embeddings, relation_embeddings, tail_embeddings, W_R, V_R, u_R, b_R):
    head = np.asarray(head_embeddings, dtype=np.float32)
    tail = np.asarray(tail_embeddings, dtype=np.float32)
    W = np.asarray(W_R, dtype=np.float32)
    V = np.asarray(V_R, dtype=np.float32)
    u = np.asarray(u_R, dtype=np.float32)
    b = np.asarray(b_R, dtype=np.float32)

    bf = ml_dtypes.bfloat16
    f8 = ml_dtypes.float8_e4m3fn

    # |u|-sorted h assignment: smallest-|u| h's take the fp8 slots
    order = np.argsort(np.abs(u), kind="stable")
    n8 = N8C * NCORES
    dr_h = order[:n8].reshape(NCORES, N8C)
    bf_h = order[n8:].reshape(NCORES, NBF)

    # [D, B] -> [P, KD, B]: partition p holds row k*128+p of the transpose
    def to_pkb(x, dt):
        return np.ascontiguousarray(
            x.T.reshape(KD, P, B).transpose(1, 0, 2).astype(dt))

    h8 = head.astype(f8).astype(np.float32)
    eh = head - h8
    hTa = to_pkb(head, bf)
    hT8a = to_pkb(head, f8)
    ehT8a = to_pkb(eh, f8)
    tTa = to_pkb(tail, bf)
    tla = tail.astype(bf)

    def to_pkd(m, dt):
        # [D, D] -> [P, KD, D]
        return np.ascontiguousarray(
            m.reshape(KD, P, D).transpose(1, 0, 2).astype(dt))

    in_maps = []
    for c in range(NCORES):
        hs = np.concatenate([dr_h[c], bf_h[c]])
        Wd = W[dr_h[c]].astype(np.float64)
        ud = u[dr_h[c]].astype(np.float64)
        W8d = Wd.astype(np.float32).astype(f8).astype(np.float64)
        Wu = np.einsum("h,hij->ij", ud, Wd)
        W8u = np.einsum("h,hij->ij", ud, W8d)
        ewu_a = to_pkd((Wu - W8u).astype(np.float32), bf)
        w8u8_a = to_pkd(W8u.astype(np.float32), f8)

        w8a = np.ascontiguousarray(
            W[dr_h[c]].reshape(N8C, KD, P, D).transpose(0, 2, 1, 3).astype(f8))
        wa = np.ascontiguousarray(
            W[bf_h[c]].reshape(NBF, KD, P, D).transpose(0, 2, 1, 3).astype(bf))
        vca = np.ascontiguousarray(
            V[hs].T.reshape(KC, P, HSL).transpose(1, 0, 2).astype(bf))
        uext = np.concatenate([u[hs], np.float32([0.5])]).astype(np.float32)
        in_maps.append({
            "hT": hTa,
            "hT8": hT8a,
            "ehT8": ehT8a,
            "tT": tTa,
            "tl": tla,
            "w": wa,
            "w8": w8a,
            "ewu": ewu_a,
            "w8u8": w8u8_a,
            "vc": vca,
            "ub": np.ascontiguousarray(np.broadcast_to(uext, (P, NCOL))),
            "br": np.ascontiguousarray(np.tile(b[hs], 4).reshape(P, 1)),
        })

    nc = _get_nc()
    trace = bool(int(os.environ.get("BILINEAR_TRACE", "0")))
    res = bass_utils.run_bass_kernel_spmd(
        nc, in_maps, core_ids=list(range(NCORES)), trace=trace
    )
    global LAST_RESULT
    LAST_RESULT = res
    if trace:
        print(f"HW exec time: {res.exec_time_ns} ns")
        if res.instructions_and_trace:
            print(f"trace: {res.instructions_and_trace[1]}")

    acc = np.zeros(B, dtype=np.float64)
    for c in range(NCORES):
        part = res.results[c]["scores_part"]  # [P, BT]
        acc += part.T.reshape(-1).astype(np.float64)
    return acc.astype(np.float32)


# revision 7
# speedup vs baseline: 1.1835x; 1.1102x over previous
"""Trainium2 Bass kernel for BilinearScoringFunction.

scores[b] = relu( einsum('bi,hij,bj->bh', head, W_R, tail)
                  + concat(head, tail) @ V_R.T + b_R ) @ u_R

B=4096, D=512, H=256. Sharded over 8 NeuronCores along the hidden dim H
(32 hidden units per core); the host sums the 8 partial score vectors.

Precision plan (validated against the reference in fp64 simulation):
  - h's are assigned to cores by |u_h| rank: the 144 smallest-|u| h's run
    as fp8-e4m3 DoubleRow matmuls (2x PE rate; 18 per core), the 112
    largest-|u| h's in bf16 (14 per core). Since each h's score-error
    contribution scales with u_h^2, sorting buys ~4.5x more fp8 h's than
    the baseline random assignment at the same error.
  - a relu mean-correction: relu(x) = x/2 + |x|/2, and the linear half of
    the fp8 quantization error collapses over h:
      sum_h u_h * (bil_h - bil8_h) = head@EWu@tail + eh@W8u@tail
    with EWu = sum u_h (W_h - W8_h) (bf16), eh = head - fp8(head) (fp8),
    W8u = sum u_h W8_h (fp8). Computed as one extra pseudo-h column
    (4 bf16 + 2 DR matmuls into one PSUM + one TTR), weighted +0.5 in the
    u-dot and excluded from the relu. Cuts the fp8 error ~sqrt(2)x.
  Simulated rel_l2 = 1.37e-2 (gate 2e-2).

Per core structure (PE ~690us, Vector ~745us -> Vector-bound):
  per h: T_h = head @ W_h on TensorE (4 accumulating K=128 bf16 matmuls
  or 2 K=256 fp8-DR matmuls, N=512), then one fused VectorE custom-DVE
  TENSOR_TENSOR_REDUCE computes bil[:, h] = rowsum(T_h * tail) straight
  out of PSUM. Linear term via h-major V@concat^T matmuls + 32x32 DVE
  transposes (as in the earlier revision). Phase 3 (add lin, relu on the
  32 real columns, u-dot over 33 columns) interleaved with the last two
  h's, bt-major.
"""

import os
from contextlib import ExitStack

import numpy as np
import ml_dtypes

import concourse.bacc as bacc
import concourse.tile as tile
import concourse.mybir as mybir
from concourse import bass_utils
from concourse.dve_ops import TENSOR_TENSOR_REDUCE

B, D, H = 4096, 512, 256
NCORES = 8
HSL = H // NCORES          # hidden units per core = 32
N8C = 21                   # fp8-DR h's per core (slots 0..N8C-1)
NBF = HSL - N8C            # bf16 h's per core (slots N8C..31)
NCOL = HSL + 1             # bil columns incl. the correction column
P = 128                    # partitions
BT = B // P                # batch tiles of 128 = 32
KD = D // P                # contraction chunks per operand = 4
KC = 2 * KD                # concat contraction chunks = 8
LIN_AT = 3                 # insert linear-term matmuls after this many
                           # h-loop rounds of the main sequence

_F32 = mybir.dt.float32
_BF16 = mybir.dt.bfloat16
_F8 = mybir.dt.float8e4

_NC_CACHE = None


def _build_nc():
    nc = bacc.Bacc(
        "TRN2",
        target_bir_lowering=False,
        debug=False,
        enable_asserts=False,
        num_devices=NCORES,
    )
    # all pre-arranged host-side so every DMA is a clean 2D/3D copy
    hT = nc.dram_tensor("hT", [P, KD, B], _BF16, kind="ExternalInput").ap()
    hT8 = nc.dram_tensor("hT8", [P, KD, B], _F8, kind="ExternalInput").ap()
    ehT8 = nc.dram_tensor("ehT8", [P, KD, B], _F8, kind="ExternalInput").ap()
    tT = nc.dram_tensor("tT", [P, KD, B], _BF16, kind="ExternalInput").ap()
    tl = nc.dram_tensor("tl", [B, D], _BF16, kind="ExternalInput").ap()
    w = nc.dram_tensor("w", [NBF, P, KD, D], _BF16, kind="ExternalInput").ap()
    w8 = nc.dram_tensor("w8", [N8C, P, KD, D], _F8, kind="ExternalInput").ap()
    ewu = nc.dram_tensor("ewu", [P, KD, D], _BF16, kind="ExternalInput").ap()
    w8u8 = nc.dram_tensor("w8u8", [P, KD, D], _F8, kind="ExternalInput").ap()
    vc = nc.dram_tensor("vc", [P, KC, HSL], _BF16, kind="ExternalInput").ap()
    ub = nc.dram_tensor("ub", [P, NCOL], _F32, kind="ExternalInput").ap()
    br = nc.dram_tensor("br", [P, 1], _F32, kind="ExternalInput").ap()
    out = nc.dram_tensor("scores_part", [P, BT], _F32, kind="ExternalOutput").ap()

    with tile.TileContext(nc) as tc, ExitStack() as ctx:
        const = ctx.enter_context(tc.tile_pool(name="const", bufs=1))
        wp = ctx.enter_context(tc.tile_pool(name="w", bufs=4))
        w8p = ctx.enter_context(tc.tile_pool(name="w8", bufs=6))
        psp = ctx.enter_context(tc.tile_pool(name="ps", bufs=8, space="PSUM"))
        scr = ctx.enter_context(tc.tile_pool(name="scr", bufs=2))

        # processing order of per-core h slots: two bf16 h's first (they
        # pace the hT/tl stream landing), then rounds interleaving DR and
        # bf16 h's per batch tile so the 256-col DR weight loads (no FWL,
        # ~213ns each) hide under bf16 matmul streams; the last two bf16
        # slots are merged with phase 3.
        first2 = [N8C, N8C + 1]
        drs = list(range(N8C))
        bfs = list(range(N8C + 2, HSL - 2))
        rounds = []
        di = 0
        for bslot in bfs:
            take = min(3, len(drs) - di)
            rounds.append(tuple(drs[di:di + take]) + (bslot,))
            di += take
        while di < len(drs):
            rounds.append(tuple(drs[di:di + 2]))
            di += 2
        last2 = [HSL - 2, HSL - 1]

        def is_f8(h):
            return h < N8C

        w_tiles = {}

        def load_w(h):
            if is_f8(h):
                t = w8p.tile([P, KD, D], _F8, name="w8t")
                nc.sync.dma_start(t[:], w8[h])
            else:
                t = wp.tile([P, KD, D], _BF16, name="wt")
                nc.sync.dma_start(t[:], w[h - N8C])
            w_tiles[h] = t
            return t

        # --- DMAs in priority order: compute start gates on w[first2[0]] + hT.
        w_tiles[first2[0]] = wp.tile([P, KD, D], _BF16, name="wt")
        nc.sync.dma_start(w_tiles[first2[0]][:, 0:2, :], w[first2[0] - N8C][:, 0:2, :])
        nc.sync.dma_start(w_tiles[first2[0]][:, 2:4, :], w[first2[0] - N8C][:, 2:4, :])

        # head^T / tail as interleaved 2-batch-tile chunks: the first-h matmul
        # quad for tile bt only gates on its 256KB hT chunk, so compute
        # starts early and the first two h's run DMA-paced as streams land.
        hT_t = const.tile([P, KD, B], _BF16)
        tT_t = const.tile([P, KD, B], _BF16)
        tl_t = const.tile([P, BT, D], _BF16)
        nc.scalar.dma_start(hT_t[:, :, 0:P], hT[:, :, 0:P])
        nc.scalar.dma_start(tl_t[:, 0:1, :], tl[0:P, :].rearrange(
            "(t p) d -> p t d", p=P))
        load_w(first2[1])
        nc.sync.dma_start(hT_t[:, :, P:2 * P], hT[:, :, P:2 * P])
        nc.sync.dma_start(tl_t[:, 1:2, :], tl[P:2 * P, :].rearrange(
            "(t p) d -> p t d", p=P))
        for bt in range(2, BT, 2):
            sl = slice(bt * P, (bt + 2) * P)
            nc.sync.dma_start(hT_t[:, :, sl], hT[:, :, sl])
            nc.sync.dma_start(tl_t[:, bt:bt + 2, :], tl[sl, :].rearrange(
                "(t p) d -> p t d", p=P))
            if bt == 8:
                # correction + first mid-seq weights: late enough not to
                # displace the stream chunks, early enough to land before use
                ewu_t = const.tile([P, KD, D], _BF16)
                nc.sync.dma_start(ewu_t[:], ewu[:])
                w8u8_t = const.tile([P, KD, D], _F8)
                nc.sync.dma_start(w8u8_t[:], w8u8[:])
                for h in rounds[0]:
                    load_w(h)
        # bulk tensors needed later: fp8 head copies, tail^T (lin phase)
        hT8_t = const.tile([P, KD, B], _F8)
        nc.sync.dma_start(hT8_t[:], hT8[:])
        ehT8_t = const.tile([P, KD, B], _F8)
        nc.sync.dma_start(ehT8_t[:], ehT8[:])
        nc.sync.dma_start(tT_t[:], tT[:])

        vc_t = const.tile([P, KC, HSL], _BF16)
        nc.sync.dma_start(vc_t[:], vc[:])
        ub_t = const.tile([P, NCOL], _F32)
        nc.sync.dma_start(ub_t[:], ub[:, :])
        br_t = const.tile([P, 1], _F32)
        nc.sync.dma_start(br_t[:], br[:, :])

        bil_t = const.tile([P, BT, NCOL], _F32)  # bilinear + corr col, b-major
        linb_t = const.tile([P, BT, HSL], _F32)  # linear + bias, b-major
        scores_t = const.tile([P, BT], _F32)

        lsp = ctx.enter_context(tc.tile_pool(name="lst", bufs=2))

        def lin_phase():
            # col-tiled: 4 batch-512 tiles accumulate concurrently in the
            # four 32-partition column groups of one PSUM bank. pl[32j+c, n]
            # = lin^T[h=c, b=(rnd*4+j)*512+n].
            for rnd in range(2):
                pl = psp.tile([P, 512], _F32, name="ps")
                for kc in range(KC):
                    for j in range(4):
                        b512 = rnd * 4 + j
                        if kc < KD:
                            rhs = hT_t[:, kc, b512 * 512:(b512 + 1) * 512]
                        else:
                            rhs = tT_t[:, kc - KD, b512 * 512:(b512 + 1) * 512]
                        nc.tensor.matmul(
                            pl[32 * j:32 * (j + 1), :], vc_t[:, kc, :], rhs,
                            start=(kc == 0), stop=(kc == KC - 1),
                            tile_position=(0, 32 * j),
                        )
                nc.vector.tensor_scalar_add(pl[:], pl[:], br_t[:])
                lin_stage = lsp.tile([P, 512], _F32, name="lst")
                nc.vector.transpose(lin_stage[:], pl[:])
                for j in range(4):
                    blk = lin_stage[32 * j:32 * (j + 1), :].rearrange(
                        "p (q m c) -> p q m c", q=4, m=4
                    )
                    for m in range(4):
                        dst = linb_t[32 * m:32 * (m + 1),
                                     (rnd * 4 + j) * 4:(rnd * 4 + j) * 4 + 4, :]
                        nc.sync.dma_start(dst, blk[:, :, m, :])

        s2p = ctx.enter_context(tc.tile_pool(name="s2", bufs=2))

        def _udot(bt):
            # scores_part[b] = (relu(bil + lin) ++ corr)[b, :] @ (u ++ 0.5)
            s2_t = s2p.tile([P, NCOL], _F32, name="s2")
            nc.vector._custom_dve(
                TENSOR_TENSOR_REDUCE,
                out=s2_t[:],
                in0=bil_t[:, bt, :],
                in1=ub_t[:],
                s0=0.0,
                s1=1.0,
                accum_out=scores_t[:, bt:bt + 1],
            )

        def _ttr(ps_t, bt, col):
            s_t = scr.tile([P, D], _F32, name="s")
            nc.vector._custom_dve(
                TENSOR_TENSOR_REDUCE,
                out=s_t[:],
                in0=ps_t[:],
                in1=tl_t[:, bt, :],
                s0=0.0,
                s1=1.0,
                accum_out=bil_t[:, bt, col:col + 1],
            )

        def quad(h, bt, w_t):
            ps_t = psp.tile([P, D], _F32, name="ps")
            if is_f8(h):
                for c in range(2):
                    nc.tensor.matmul(
                        ps_t[:],
                        hT8_t[:, 2 * c:2 * c + 2, bt * P:(bt + 1) * P],
                        w_t[:, 2 * c:2 * c + 2, :],
                        start=(c == 0),
                        stop=(c == 1),
                        perf_mode=mybir.MatmulPerfMode.DoubleRow,
                    )
            else:
                for k in range(KD):
                    nc.tensor.matmul(
                        ps_t[:],
                        hT_t[:, k, bt * P:(bt + 1) * P],
                        w_t[:, k, :],
                        start=(k == 0),
                        stop=(k == KD - 1),
                    )
            _ttr(ps_t, bt, h)

        def corr_quad(bt):
            # bil[:, 32] = head@EWu@tail + eh8@W8u8@tail  (= sum_h u_h * fp8
            # quantization error over the DR h's, udot weight +0.5)
            ps_t = psp.tile([P, D], _F32, name="ps")
            for k in range(KD):
                nc.tensor.matmul(
                    ps_t[:],
                    hT_t[:, k, bt * P:(bt + 1) * P],
                    ewu_t[:, k, :],
                    start=(k == 0), stop=False,
                )
            for c in range(2):
                nc.tensor.matmul(
                    ps_t[:],
                    ehT8_t[:, 2 * c:2 * c + 2, bt * P:(bt + 1) * P],
                    w8u8_t[:, 2 * c:2 * c + 2, :],
                    start=False, stop=(c == 1),
                    perf_mode=mybir.MatmulPerfMode.DoubleRow,
                )
            _ttr(ps_t, bt, HSL)

        # --- main sequence ---
        # first two bf16 h's interleaved per bt while streams land
        for bt in range(BT):
            quad(first2[0], bt, w_tiles[first2[0]])
            quad(first2[1], bt, w_tiles[first2[1]])
        w_tiles.pop(first2[0])
        w_tiles.pop(first2[1])

        # correction column (needs hT fully landed + ewu/w8u8/ehT8)
        for bt in range(BT):
            corr_quad(bt)

        for ridx, rnd in enumerate(rounds):
            # prefetch next round's weights (round ~56us, DMA ~2-8us)
            if ridx + 1 < len(rounds):
                for h in rounds[ridx + 1]:
                    load_w(h)
            else:
                for h in last2:
                    load_w(h)
            tiles = [w_tiles.pop(h) for h in rnd]
            for bt in range(BT):
                for h, w_t in zip(rnd, tiles):
                    quad(h, bt, w_t)
            if ridx == LIN_AT - 1:
                lin_phase()

        # last two h's merged bt-major with phase 3
        h30, h31 = last2
        w30 = w_tiles.pop(h30)
        w31 = w_tiles.pop(h31)
        for bt in range(BT):
            quad(h30, bt, w30)
            quad(h31, bt, w31)
            # in-place: bil := relu(bil + lin) on the 32 real columns
            nc.gpsimd.tensor_tensor(
                out=bil_t[:, bt, 0:HSL], in0=bil_t[:, bt, 0:HSL],
                in1=linb_t[:, bt, :], op=mybir.AluOpType.add,
            )
            if bt == BT - 1:
                nc.vector.tensor_scalar_max(
                    bil_t[:, bt, 0:HSL], bil_t[:, bt, 0:HSL], 0.0
                )
            else:
                nc.scalar.activation(
                    bil_t[:, bt, 0:HSL], bil_t[:, bt, 0:HSL],
                    mybir.ActivationFunctionType.Relu,
                )
            if bt >= 1:
                _udot(bt - 1)
            if bt - 1 == 15:
                nc.sync.dma_start(out[:, 0:16], scores_t[:, 0:16])
            if bt - 1 == 30:
                nc.sync.dma_start(out[:, 16:31], scores_t[:, 16:31])

        _udot(BT - 1)
        nc.sync.dma_start(out[:, 31:BT], scores_t[:, 31:BT])

    nc.compile()
    return nc


def _get_nc():
    global _NC_CACHE
    if _NC_CACHE is None:
        _NC_CACHE = _build_nc()
    return _NC_CACHE


def kernel(head_embeddings, relation_embeddings, tail_embeddings, W_R, V_R, u_R, b_R):
    head = np.asarray(head_embeddings, dtype=np.float32)
    tail = np.asarray(tail_embeddings, dtype=np.float32)
    W = np.asarray(W_R, dtype=np.float32)
    V = np.asarray(V_R, dtype=np.float32)
    u = np.asarray(u_R, dtype=np.float32)
    b = np.asarray(b_R, dtype=np.float32)

    bf = ml_dtypes.bfloat16
    f8 = ml_dtypes.float8_e4m3fn

    # |u|-sorted h assignment: smallest-|u| h's take the fp8 slots
    order = np.argsort(np.abs(u), kind="stable")
    n8 = N8C * NCORES
    dr_h = order[:n8].reshape(NCORES, N8C)
    bf_h = order[n8:].reshape(NCORES, NBF)

    # [D, B] -> [P, KD, B]: partition p holds row k*128+p of the transpose
    def to_pkb(x, dt):
        return np.ascontiguousarray(
            x.T.reshape(KD, P, B).transpose(1, 0, 2).astype(dt))

    h8 = head.astype(f8).astype(np.float32)
    eh = head - h8
    hTa = to_pkb(head, bf)
    hT8a = to_pkb(head, f8)
    ehT8a = to_pkb(eh, f8)
    tTa = to_pkb(tail, bf)
    tla = tail.astype(bf)

    def to_pkd(m, dt):
        # [D, D] -> [P, KD, D]
        return np.ascontiguousarray(
            m.reshape(KD, P, D).transpose(1, 0, 2).astype(dt))

    in_maps = []
    for c in range(NCORES):
        hs = np.concatenate([dr_h[c], bf_h[c]])
        Wd = W[dr_h[c]].astype(np.float64)
        ud = u[dr_h[c]].astype(np.float64)
        W8d = Wd.astype(np.float32).astype(f8).astype(np.float64)
        Wu = np.einsum("h,hij->ij", ud, Wd)
        W8u = np.einsum("h,hij->ij", ud, W8d)
        ewu_a = to_pkd((Wu - W8u).astype(np.float32), bf)
        w8u8_a = to_pkd(W8u.astype(np.float32), f8)

        w8a = np.ascontiguousarray(
            W[dr_h[c]].reshape(N8C, KD, P, D).transpose(0, 2, 1, 3).astype(f8))
        wa = np.ascontiguousarray(
            W[bf_h[c]].reshape(NBF, KD, P, D).transpose(0, 2, 1, 3).astype(bf))
        vca = np.ascontiguousarray(
            V[hs].T.reshape(KC, P, HSL).transpose(1, 0, 2).astype(bf))
        uext = np.concatenate([u[hs], np.float32([0.5])]).astype(np.float32)
        in_maps.append({
            "hT": hTa,
            "hT8": hT8a,
            "ehT8": ehT8a,
            "tT": tTa,
            "tl": tla,
            "w": wa,
            "w8": w8a,
            "ewu": ewu_a,
            "w8u8": w8u8_a,
            "vc": vca,
            "ub": np.ascontiguousarray(np.broadcast_to(uext, (P, NCOL))),
            "br": np.ascontiguousarray(np.tile(b[hs], 4).reshape(P, 1)),
        })

    nc = _get_nc()
    trace = bool(int(os.environ.get("BILINEAR_TRACE", "0")))
    res = bass_utils.run_bass_kernel_spmd(
        nc, in_maps, core_ids=list(range(NCORES)), trace=trace
    )
    global LAST_RESULT
    LAST_RESULT = res
    if trace:
        print(f"HW exec time: {res.exec_time_ns} ns")
        if res.instructions_and_trace:
            print(f"trace: {res.instructions_and_trace[1]}")

    acc = np.zeros(B, dtype=np.float64)
    for c in range(NCORES):
        part = res.results[c]["scores_part"]  # [P, BT]
        acc += part.T.reshape(-1).astype(np.float64)
    return acc.astype(np.float32)


# revision 8
# speedup vs baseline: 1.2012x; 1.0150x over previous
"""Trainium2 Bass kernel for BilinearScoringFunction.

scores[b] = relu( einsum('bi,hij,bj->bh', head, W_R, tail)
                  + concat(head, tail) @ V_R.T + b_R ) @ u_R

B=4096, D=512, H=256. Sharded over 8 NeuronCores along the hidden dim H
(32 hidden units per core); the host sums the 8 partial score vectors.

Precision plan (validated against the reference in fp64 simulation):
  - h's are assigned to cores by |u_h| rank: the 144 smallest-|u| h's run
    as fp8-e4m3 DoubleRow matmuls (2x PE rate; 18 per core), the 112
    largest-|u| h's in bf16 (14 per core). Since each h's score-error
    contribution scales with u_h^2, sorting buys ~4.5x more fp8 h's than
    the baseline random assignment at the same error.
  - a relu mean-correction: relu(x) = x/2 + |x|/2, and the linear half of
    the fp8 quantization error collapses over h:
      sum_h u_h * (bil_h - bil8_h) = head@EWu@tail + eh@W8u@tail
    with EWu = sum u_h (W_h - W8_h) (bf16), eh = head - fp8(head) (fp8),
    W8u = sum u_h W8_h (fp8). Computed as one extra pseudo-h column
    (4 bf16 + 2 DR matmuls into one PSUM + one TTR), weighted +0.5 in the
    u-dot and excluded from the relu. Cuts the fp8 error ~sqrt(2)x.
  Simulated rel_l2 = 1.37e-2 (gate 2e-2).

Per core structure (PE ~690us, Vector ~745us -> Vector-bound):
  per h: T_h = head @ W_h on TensorE (4 accumulating K=128 bf16 matmuls
  or 2 K=256 fp8-DR matmuls, N=512), then one fused VectorE custom-DVE
  TENSOR_TENSOR_REDUCE computes bil[:, h] = rowsum(T_h * tail) straight
  out of PSUM. Linear term via h-major V@concat^T matmuls + 32x32 DVE
  transposes (as in the earlier revision). Phase 3 (add lin, relu on the
  32 real columns, u-dot over 33 columns) interleaved with the last two
  h's, bt-major.
"""

import os
from contextlib import ExitStack

import numpy as np
import ml_dtypes

import concourse.bacc as bacc
import concourse.tile as tile
import concourse.mybir as mybir
from concourse import bass_utils
from concourse.dve_ops import TENSOR_TENSOR_REDUCE

B, D, H = 4096, 512, 256
NCORES = 8
HSL = H // NCORES          # hidden units per core = 32
N8C = 21                   # fp8-DR h's per core (slots 0..N8C-1)
NBF = HSL - N8C            # bf16 h's per core (slots N8C..31)
NCOL = HSL + 1             # bil columns incl. the correction column
P = 128                    # partitions
BT = B // P                # batch tiles of 128 = 32
KD = D // P                # contraction chunks per operand = 4
KC = 2 * KD                # concat contraction chunks = 8
LIN_AT = 3                 # insert linear-term matmuls after this many
                           # h-loop rounds of the main sequence

_F32 = mybir.dt.float32
_BF16 = mybir.dt.bfloat16
_F8 = mybir.dt.float8e4

_NC_CACHE = None


def _build_nc():
    nc = bacc.Bacc(
        "TRN2",
        target_bir_lowering=False,
        debug=False,
        enable_asserts=False,
        num_devices=NCORES,
    )
    # all pre-arranged host-side so every DMA is a clean 2D/3D copy
    hT = nc.dram_tensor("hT", [P, KD, B], _BF16, kind="ExternalInput").ap()
    hT8 = nc.dram_tensor("hT8", [P, KD, B], _F8, kind="ExternalInput").ap()
    ehT8 = nc.dram_tensor("ehT8", [P, KD, B], _F8, kind="ExternalInput").ap()
    tT = nc.dram_tensor("tT", [P, KD, B], _BF16, kind="ExternalInput").ap()
    tl = nc.dram_tensor("tl", [B, D], _BF16, kind="ExternalInput").ap()
    w = nc.dram_tensor("w", [NBF, P, KD, D], _BF16, kind="ExternalInput").ap()
    w8 = nc.dram_tensor("w8", [N8C, P, KD, D], _F8, kind="ExternalInput").ap()
    ewu = nc.dram_tensor("ewu", [P, KD, D], _BF16, kind="ExternalInput").ap()
    w8u8 = nc.dram_tensor("w8u8", [P, KD, D], _F8, kind="ExternalInput").ap()
    vc = nc.dram_tensor("vc", [P, KC, HSL], _BF16, kind="ExternalInput").ap()
    ub = nc.dram_tensor("ub", [P, NCOL], _F32, kind="ExternalInput").ap()
    br = nc.dram_tensor("br", [P, 1], _F32, kind="ExternalInput").ap()
    out = nc.dram_tensor("scores_part", [P, BT], _F32, kind="ExternalOutput").ap()

    with tile.TileContext(nc) as tc, ExitStack() as ctx:
        const = ctx.enter_context(tc.tile_pool(name="const", bufs=1))
        wp = ctx.enter_context(tc.tile_pool(name="w", bufs=4))
        w8p = ctx.enter_context(tc.tile_pool(name="w8", bufs=6))
        psp = ctx.enter_context(tc.tile_pool(name="ps", bufs=8, space="PSUM"))
        scr = ctx.enter_context(tc.tile_pool(name="scr", bufs=2))

        # processing order of per-core h slots: two bf16 h's first (they
        # pace the hT/tl stream landing), then rounds interleaving DR and
        # bf16 h's per batch tile so the 256-col DR weight loads (no FWL,
        # ~213ns each) hide under bf16 matmul streams; the last two bf16
        # slots are merged with phase 3.
        first2 = [N8C, N8C + 1]
        last2 = [N8C - 1, HSL - 1]
        drs = list(range(N8C - 1))
        bfs = list(range(N8C + 2, HSL - 1))
        rounds = []
        di = 0
        for bslot in bfs:
            rounds.append((drs[di], drs[di + 1], bslot))
            di += 2
        while di < len(drs):
            rounds.append(tuple(drs[di:di + 2]))
            di += 2

        def is_f8(h):
            return h < N8C

        w_tiles = {}

        def load_w(h):
            if is_f8(h):
                t = w8p.tile([P, KD, D], _F8, name="w8t")
                nc.sync.dma_start(t[:], w8[h])
            else:
                t = wp.tile([P, KD, D], _BF16, name="wt")
                nc.sync.dma_start(t[:], w[h - N8C])
            w_tiles[h] = t
            return t

        # --- DMAs in priority order: compute start gates on w[first2[0]] + hT.
        w_tiles[first2[0]] = wp.tile([P, KD, D], _BF16, name="wt")
        nc.sync.dma_start(w_tiles[first2[0]][:, 0:2, :], w[first2[0] - N8C][:, 0:2, :])
        nc.sync.dma_start(w_tiles[first2[0]][:, 2:4, :], w[first2[0] - N8C][:, 2:4, :])

        # head^T / tail as interleaved 2-batch-tile chunks: the first-h matmul
        # quad for tile bt only gates on its 256KB hT chunk, so compute
        # starts early and the first two h's run DMA-paced as streams land.
        hT_t = const.tile([P, KD, B], _BF16)
        tT_t = const.tile([P, KD, B], _BF16)
        tl_t = const.tile([P, BT, D], _BF16)
        nc.scalar.dma_start(hT_t[:, 0:2, 0:P], hT[:, 0:2, 0:P])
        nc.scalar.dma_start(hT_t[:, 2:4, 0:P], hT[:, 2:4, 0:P])
        nc.scalar.dma_start(tl_t[:, 0:1, :], tl[0:P, :].rearrange(
            "(t p) d -> p t d", p=P))
        load_w(first2[1])
        nc.sync.dma_start(hT_t[:, :, P:2 * P], hT[:, :, P:2 * P])
        nc.sync.dma_start(tl_t[:, 1:2, :], tl[P:2 * P, :].rearrange(
            "(t p) d -> p t d", p=P))
        for bt in range(2, BT, 2):
            sl = slice(bt * P, (bt + 2) * P)
            nc.sync.dma_start(hT_t[:, :, sl], hT[:, :, sl])
            nc.sync.dma_start(tl_t[:, bt:bt + 2, :], tl[sl, :].rearrange(
                "(t p) d -> p t d", p=P))
            if bt == 8:
                # correction + first mid-seq weights: late enough not to
                # displace the stream chunks, early enough to land before use
                ewu_t = const.tile([P, KD, D], _BF16)
                nc.sync.dma_start(ewu_t[:], ewu[:])
                w8u8_t = const.tile([P, KD, D], _F8)
                nc.sync.dma_start(w8u8_t[:], w8u8[:])
                for h in rounds[0]:
                    load_w(h)
        # bulk tensors needed later: fp8 head copies, tail^T (lin phase)
        hT8_t = const.tile([P, KD, B], _F8)
        nc.sync.dma_start(hT8_t[:], hT8[:])
        ehT8_t = const.tile([P, KD, B], _F8)
        nc.sync.dma_start(ehT8_t[:], ehT8[:])
        nc.sync.dma_start(tT_t[:], tT[:])

        vc_t = const.tile([P, KC, HSL], _BF16)
        nc.sync.dma_start(vc_t[:], vc[:])
        ub_t = const.tile([P, NCOL], _F32)
        nc.sync.dma_start(ub_t[:], ub[:, :])
        br_t = const.tile([P, 1], _F32)
        nc.sync.dma_start(br_t[:], br[:, :])

        bil_t = const.tile([P, BT, NCOL], _F32)  # bilinear + corr col, b-major
        linb_t = const.tile([P, BT, HSL], _F32)  # linear + bias, b-major
        scores_t = const.tile([P, BT], _F32)

        lsp = ctx.enter_context(tc.tile_pool(name="lst", bufs=2))

        def lin_phase():
            # col-tiled: 4 batch-512 tiles accumulate concurrently in the
            # four 32-partition column groups of one PSUM bank. pl[32j+c, n]
            # = lin^T[h=c, b=(rnd*4+j)*512+n].
            for rnd in range(2):
                pl = psp.tile([P, 512], _F32, name="ps")
                for kc in range(KC):
                    for j in range(4):
                        b512 = rnd * 4 + j
                        if kc < KD:
                            rhs = hT_t[:, kc, b512 * 512:(b512 + 1) * 512]
                        else:
                            rhs = tT_t[:, kc - KD, b512 * 512:(b512 + 1) * 512]
                        nc.tensor.matmul(
                            pl[32 * j:32 * (j + 1), :], vc_t[:, kc, :], rhs,
                            start=(kc == 0), stop=(kc == KC - 1),
                            tile_position=(0, 32 * j),
                        )
                nc.scalar.activation(
                    pl[:], pl[:], mybir.ActivationFunctionType.Identity,
                    bias=br_t[:],
                )
                lin_stage = lsp.tile([P, 512], _F32, name="lst")
                nc.vector.transpose(lin_stage[:], pl[:])
                for j in range(4):
                    blk = lin_stage[32 * j:32 * (j + 1), :].rearrange(
                        "p (q m c) -> p q m c", q=4, m=4
                    )
                    for m in range(4):
                        dst = linb_t[32 * m:32 * (m + 1),
                                     (rnd * 4 + j) * 4:(rnd * 4 + j) * 4 + 4, :]
                        nc.sync.dma_start(dst, blk[:, :, m, :])

        s2p = ctx.enter_context(tc.tile_pool(name="s2", bufs=2))

        def _udot(bt):
            # scores_part[b] = (relu(bil + lin) ++ corr)[b, :] @ (u ++ 0.5)
            s2_t = s2p.tile([P, NCOL], _F32, name="s2")
            nc.vector.scalar_tensor_tensor(
                out=s2_t[:],
                in0=bil_t[:, bt, :],
                scalar=1.0,
                in1=ub_t[:],
                op0=mybir.AluOpType.mult,
                op1=mybir.AluOpType.mult,
                accum_out=scores_t[:, bt:bt + 1],
            )

        def _ttr(ps_t, bt, col):
            s_t = scr.tile([P, D], _F32, name="s")
            nc.vector.scalar_tensor_tensor(
                out=s_t[:],
                in0=ps_t[:],
                scalar=1.0,
                in1=tl_t[:, bt, :],
                op0=mybir.AluOpType.mult,
                op1=mybir.AluOpType.mult,
                accum_out=bil_t[:, bt, col:col + 1],
            )

        def quad(h, bt, w_t):
            ps_t = psp.tile([P, D], _F32, name="ps")
            if is_f8(h):
                for c in range(2):
                    nc.tensor.matmul(
                        ps_t[:],
                        hT8_t[:, 2 * c:2 * c + 2, bt * P:(bt + 1) * P],
                        w_t[:, 2 * c:2 * c + 2, :],
                        start=(c == 0),
                        stop=(c == 1),
                        perf_mode=mybir.MatmulPerfMode.DoubleRow,
                    )
            else:
                for k in range(KD):
                    nc.tensor.matmul(
                        ps_t[:],
                        hT_t[:, k, bt * P:(bt + 1) * P],
                        w_t[:, k, :],
                        start=(k == 0),
                        stop=(k == KD - 1),
                    )
            _ttr(ps_t, bt, h)

        def corr_quad(bt):
            # bil[:, 32] = head@EWu@tail + eh8@W8u8@tail  (= sum_h u_h * fp8
            # quantization error over the DR h's, udot weight +0.5)
            ps_t = psp.tile([P, D], _F32, name="ps")
            for k in range(KD):
                nc.tensor.matmul(
                    ps_t[:],
                    hT_t[:, k, bt * P:(bt + 1) * P],
                    ewu_t[:, k, :],
                    start=(k == 0), stop=False,
                )
            for c in range(2):
                nc.tensor.matmul(
                    ps_t[:],
                    ehT8_t[:, 2 * c:2 * c + 2, bt * P:(bt + 1) * P],
                    w8u8_t[:, 2 * c:2 * c + 2, :],
                    start=False, stop=(c == 1),
                    perf_mode=mybir.MatmulPerfMode.DoubleRow,
                )
            _ttr(ps_t, bt, HSL)

        # --- main sequence ---
        # first two bf16 h's interleaved per bt while streams land
        for bt in range(BT):
            quad(first2[0], bt, w_tiles[first2[0]])
            quad(first2[1], bt, w_tiles[first2[1]])
        w_tiles.pop(first2[0])
        w_tiles.pop(first2[1])

        # correction column (needs hT fully landed + ewu/w8u8/ehT8)
        for bt in range(BT):
            corr_quad(bt)

        for ridx, rnd in enumerate(rounds):
            # prefetch next round's weights (round ~56us, DMA ~2-8us)
            if ridx + 1 < len(rounds):
                for h in rounds[ridx + 1]:
                    load_w(h)
            else:
                for h in last2:
                    load_w(h)
            tiles = [w_tiles.pop(h) for h in rnd]
            for bt in range(BT):
                for h, w_t in zip(rnd, tiles):
                    quad(h, bt, w_t)
            if ridx == LIN_AT - 1:
                lin_phase()

        # last two h's merged bt-major with phase 3
        h30, h31 = last2
        w30 = w_tiles.pop(h30)
        w31 = w_tiles.pop(h31)
        for bt in range(BT):
            quad(h30, bt, w30)
            quad(h31, bt, w31)
            # in-place: bil := relu(bil + lin) on the 32 real columns
            nc.gpsimd.tensor_tensor(
                out=bil_t[:, bt, 0:HSL], in0=bil_t[:, bt, 0:HSL],
                in1=linb_t[:, bt, :], op=mybir.AluOpType.add,
            )
            if bt == BT - 1:
                nc.vector.tensor_scalar_max(
                    bil_t[:, bt, 0:HSL], bil_t[:, bt, 0:HSL], 0.0
                )
            else:
                nc.scalar.activation(
                    bil_t[:, bt, 0:HSL], bil_t[:, bt, 0:HSL],
                    mybir.ActivationFunctionType.Relu,
                )
            if bt >= 1:
                _udot(bt - 1)
            if bt - 1 == 15:
                nc.sync.dma_start(out[:, 0:16], scores_t[:, 0:16])
            if bt - 1 == 30:
                nc.sync.dma_start(out[:, 16:31], scores_t[:, 16:31])

        _udot(BT - 1)
        nc.sync.dma_start(out[:, 31:BT], scores_t[:, 31:BT])

    nc.compile()
    return nc


def _get_nc():
    global _NC_CACHE
    if _NC_CACHE is None:
        _NC_CACHE = _build_nc()
    return _NC_CACHE


def kernel(head_embeddings, relation_embeddings, tail_embeddings, W_R, V_R, u_R, b_R):
    head = np.asarray(head_embeddings, dtype=np.float32)
    tail = np.asarray(tail_embeddings, dtype=np.float32)
    W = np.asarray(W_R, dtype=np.float32)
    V = np.asarray(V_R, dtype=np.float32)
    u = np.asarray(u_R, dtype=np.float32)
    b = np.asarray(b_R, dtype=np.float32)

    bf = ml_dtypes.bfloat16
    f8 = ml_dtypes.float8_e4m3fn

    # |u|-sorted h assignment: smallest-|u| h's take the fp8 slots
    order = np.argsort(np.abs(u), kind="stable")
    n8 = N8C * NCORES
    dr_h = order[:n8].reshape(NCORES, N8C)
    bf_h = order[n8:].reshape(NCORES, NBF)

    # [D, B] -> [P, KD, B]: partition p holds row k*128+p of the transpose
    def to_pkb(x, dt):
        return np.ascontiguousarray(
            x.T.reshape(KD, P, B).transpose(1, 0, 2).astype(dt))

    h8 = head.astype(f8).astype(np.float32)
    eh = head - h8
    hTa = to_pkb(head, bf)
    hT8a = to_pkb(head, f8)
    ehT8a = to_pkb(eh, f8)
    tTa = to_pkb(tail, bf)
    tla = tail.astype(bf)

    def to_pkd(m, dt):
        # [D, D] -> [P, KD, D]
        return np.ascontiguousarray(
            m.reshape(KD, P, D).transpose(1, 0, 2).astype(dt))

    in_maps = []
    for c in range(NCORES):
        hs = np.concatenate([dr_h[c], bf_h[c]])
        Wd = W[dr_h[c]].astype(np.float64)
        ud = u[dr_h[c]].astype(np.float64)
        W8d = Wd.astype(np.float32).astype(f8).astype(np.float64)
        Wu = np.einsum("h,hij->ij", ud, Wd)
        W8u = np.einsum("h,hij->ij", ud, W8d)
        ewu_a = to_pkd((Wu - W8u).astype(np.float32), bf)
        w8u8_a = to_pkd(W8u.astype(np.float32), f8)

        w8a = np.ascontiguousarray(
            W[dr_h[c]].reshape(N8C, KD, P, D).transpose(0, 2, 1, 3).astype(f8))
        wa = np.ascontiguousarray(
            W[bf_h[c]].reshape(NBF, KD, P, D).transpose(0, 2, 1, 3).astype(bf))
        vca = np.ascontiguousarray(
            V[hs].T.reshape(KC, P, HSL).transpose(1, 0, 2).astype(bf))
        uext = np.concatenate([u[hs], np.float32([0.5])]).astype(np.float32)
        in_maps.append({
            "hT": hTa,
            "hT8": hT8a,
            "ehT8": ehT8a,
            "tT": tTa,
            "tl": tla,
            "w": wa,
            "w8": w8a,
            "ewu": ewu_a,
            "w8u8": w8u8_a,
            "vc": vca,
            "ub": np.ascontiguousarray(np.broadcast_to(uext, (P, NCOL))),
            "br": np.ascontiguousarray(np.tile(b[hs], 4).reshape(P, 1)),
        })

    nc = _get_nc()
    trace = bool(int(os.environ.get("BILINEAR_TRACE", "0")))
    res = bass_utils.run_bass_kernel_spmd(
        nc, in_maps, core_ids=list(range(NCORES)), trace=trace
    )
    global LAST_RESULT
    LAST_RESULT = res
    if trace:
        print(f"HW exec time: {res.exec_time_ns} ns")
        if res.instructions_and_trace:
            print(f"trace: {res.instructions_and_trace[1]}")

    acc = np.zeros(B, dtype=np.float64)
    for c in range(NCORES):
        part = res.results[c]["scores_part"]  # [P, BT]
        acc += part.T.reshape(-1).astype(np.float64)
    return acc.astype(np.float32)


# revision 9
# speedup vs baseline: 1.2013x; 1.0001x over previous
"""Trainium2 Bass kernel for BilinearScoringFunction.

scores[b] = relu( einsum('bi,hij,bj->bh', head, W_R, tail)
                  + concat(head, tail) @ V_R.T + b_R ) @ u_R

B=4096, D=512, H=256. Sharded over 8 NeuronCores along the hidden dim H
(32 hidden units per core); the host sums the 8 partial score vectors.

Precision plan (validated against the reference in fp64 simulation, which
reproduces the measured rel_err to 4 digits):
  - h's are assigned to cores by |u_h| rank: the 168 smallest-|u| h's run
    as fp8-e4m3 DoubleRow matmuls (2x PE rate; 21 per core), the 88
    largest-|u| h's in bf16 (11 per core). Since each h's score-error
    contribution scales with u_h^2, u-sorting buys ~5x more fp8 h's than
    a random assignment at the same error (bottom-n of 256 uniform |u|
    carries only (n/256)^2 of the average u^2 weight).
  - a relu mean-correction: relu(x) = x/2 + |x|/2, and the linear half of
    the fp8 quantization error collapses over h:
      sum_h u_h * (bil_h - bil8_h) ~= head@EWu@tail + eh8@W8u8@tail
    with EWu = sum u_h (W_h - W8_h) (bf16, small entries), eh8 =
    fp8(head - fp8(head)), W8u8 = fp8(sum u_h W8_h). Computed as one
    extra pseudo-h column (4 bf16 + 2 DR matmuls into one PSUM + one
    reduce), weighted +0.5 in the u-dot and excluded from the relu.
    Cuts the fp8 error ~sqrt(2)x. Measured rel_l2 = 1.68e-2 (gate 2e-2).

Engine balance (measured): Vector 693us busy / PE 682us busy over a
~711us span -- both saturated. The per-(h,bt) reduce is the hard floor:
every fused 2-stream DVE op (custom TTR, native TTR, STT) runs at 1
elem/cycle/partition regardless of dtype on this RTL (2x_1p only
engages for 1-stream ops), so 32 h x 32 bt x 512 cols ~= 600ns each.

Per core structure:
  per h: T_h = head @ W_h on TensorE (4 accumulating K=128 bf16 matmuls
  or 2 K=256 fp8-DR matmuls, N=512), then one native VectorE
  scalar_tensor_tensor computes bil[:, h] = rowsum(T_h * tail) straight
  out of PSUM (accum_out). h's processed in (DR,DR,BF) rounds
  interleaved per batch tile so the 256-col DR weight loads (no FWL,
  ~213ns each) hide under bf16 matmul streams. Linear term via h-major
  V@concat^T matmuls + 32x32 DVE transposes, bias added on ScalarE.
  Phase 3 (add lin on GpSimd, relu on ScalarE, u-dot over 33 columns on
  VectorE) interleaved with the last two h's, bt-major.
"""

import os
from contextlib import ExitStack

import numpy as np
import ml_dtypes

import concourse.bacc as bacc
import concourse.tile as tile
import concourse.mybir as mybir
from concourse import bass_utils

B, D, H = 4096, 512, 256
NCORES = 8
HSL = H // NCORES          # hidden units per core = 32
N8C = 21                   # fp8-DR h's per core (slots 0..N8C-1)
NBF = HSL - N8C            # bf16 h's per core (slots N8C..31)
NCOL = HSL + 1             # bil columns incl. the correction column
P = 128                    # partitions
BT = B // P                # batch tiles of 128 = 32
KD = D // P                # contraction chunks per operand = 4
KC = 2 * KD                # concat contraction chunks = 8
LIN_AT = 3                 # insert linear-term matmuls after this many
                           # h-loop rounds of the main sequence

_F32 = mybir.dt.float32
_BF16 = mybir.dt.bfloat16
_F8 = mybir.dt.float8e4

_NC_CACHE = None


def _build_nc():
    nc = bacc.Bacc(
        "TRN2",
        target_bir_lowering=False,
        debug=False,
        enable_asserts=False,
        num_devices=NCORES,
    )
    # all pre-arranged host-side so every DMA is a clean 2D/3D copy
    hT = nc.dram_tensor("hT", [P, KD, B], _BF16, kind="ExternalInput").ap()
    hT8 = nc.dram_tensor("hT8", [P, KD, B], _F8, kind="ExternalInput").ap()
    ehT8 = nc.dram_tensor("ehT8", [P, KD, B], _F8, kind="ExternalInput").ap()
    tT = nc.dram_tensor("tT", [P, KD, B], _BF16, kind="ExternalInput").ap()
    tl = nc.dram_tensor("tl", [B, D], _BF16, kind="ExternalInput").ap()
    w = nc.dram_tensor("w", [NBF, P, KD, D], _BF16, kind="ExternalInput").ap()
    w8 = nc.dram_tensor("w8", [N8C, P, KD, D], _F8, kind="ExternalInput").ap()
    ewu = nc.dram_tensor("ewu", [P, KD, D], _BF16, kind="ExternalInput").ap()
    w8u8 = nc.dram_tensor("w8u8", [P, KD, D], _F8, kind="ExternalInput").ap()
    vc = nc.dram_tensor("vc", [P, KC, HSL], _BF16, kind="ExternalInput").ap()
    ub = nc.dram_tensor("ub", [P, NCOL], _F32, kind="ExternalInput").ap()
    br = nc.dram_tensor("br", [P, 1], _F32, kind="ExternalInput").ap()
    out = nc.dram_tensor("scores_part", [P, BT], _F32, kind="ExternalOutput").ap()

    with tile.TileContext(nc) as tc, ExitStack() as ctx:
        const = ctx.enter_context(tc.tile_pool(name="const", bufs=1))
        wp = ctx.enter_context(tc.tile_pool(name="w", bufs=4))
        w8p = ctx.enter_context(tc.tile_pool(name="w8", bufs=6))
        psp = ctx.enter_context(tc.tile_pool(name="ps", bufs=8, space="PSUM"))
        scr = ctx.enter_context(tc.tile_pool(name="scr", bufs=2))

        # processing order of per-core h slots: two bf16 h's first (they
        # pace the hT/tl stream landing), then rounds interleaving DR and
        # bf16 h's per batch tile so the 256-col DR weight loads (no FWL,
        # ~213ns each) hide under bf16 matmul streams; the last two bf16
        # slots are merged with phase 3.
        first2 = [N8C, N8C + 1]
        last2 = [N8C - 1, HSL - 1]
        drs = list(range(N8C - 1))
        bfs = list(range(N8C + 2, HSL - 1))
        rounds = []
        di = 0
        for bslot in bfs:
            rounds.append((drs[di], drs[di + 1], bslot))
            di += 2
        while di < len(drs):
            rounds.append(tuple(drs[di:di + 2]))
            di += 2

        def is_f8(h):
            return h < N8C

        w_tiles = {}

        def load_w(h):
            if is_f8(h):
                t = w8p.tile([P, KD, D], _F8, name="w8t")
                nc.sync.dma_start(t[:], w8[h])
            else:
                t = wp.tile([P, KD, D], _BF16, name="wt")
                nc.sync.dma_start(t[:], w[h - N8C])
            w_tiles[h] = t
            return t

        # --- DMAs in priority order: compute start gates on w[first2[0]] + hT.
        w_tiles[first2[0]] = wp.tile([P, KD, D], _BF16, name="wt")
        nc.sync.dma_start(w_tiles[first2[0]][:, 0:2, :], w[first2[0] - N8C][:, 0:2, :])
        nc.sync.dma_start(w_tiles[first2[0]][:, 2:4, :], w[first2[0] - N8C][:, 2:4, :])

        # head^T / tail as interleaved 2-batch-tile chunks: the first-h matmul
        # quad for tile bt only gates on its 256KB hT chunk, so compute
        # starts early and the first two h's run DMA-paced as streams land.
        hT_t = const.tile([P, KD, B], _BF16)
        tT_t = const.tile([P, KD, B], _BF16)
        tl_t = const.tile([P, BT, D], _BF16)
        nc.scalar.dma_start(hT_t[:, 0:2, 0:P], hT[:, 0:2, 0:P])
        nc.scalar.dma_start(hT_t[:, 2:4, 0:P], hT[:, 2:4, 0:P])
        nc.scalar.dma_start(tl_t[:, 0:1, :], tl[0:P, :].rearrange(
            "(t p) d -> p t d", p=P))
        load_w(first2[1])
        nc.sync.dma_start(hT_t[:, :, P:2 * P], hT[:, :, P:2 * P])
        nc.sync.dma_start(tl_t[:, 1:2, :], tl[P:2 * P, :].rearrange(
            "(t p) d -> p t d", p=P))
        for bt in range(2, BT, 2):
            sl = slice(bt * P, (bt + 2) * P)
            nc.sync.dma_start(hT_t[:, :, sl], hT[:, :, sl])
            nc.sync.dma_start(tl_t[:, bt:bt + 2, :], tl[sl, :].rearrange(
                "(t p) d -> p t d", p=P))
            if bt == 8:
                # correction + first mid-seq weights: late enough not to
                # displace the stream chunks, early enough to land before use
                ewu_t = const.tile([P, KD, D], _BF16)
                nc.sync.dma_start(ewu_t[:], ewu[:])
                w8u8_t = const.tile([P, KD, D], _F8)
                nc.sync.dma_start(w8u8_t[:], w8u8[:])
                for h in rounds[0]:
                    load_w(h)
        # bulk tensors needed later: fp8 head copies, tail^T (lin phase)
        hT8_t = const.tile([P, KD, B], _F8)
        nc.sync.dma_start(hT8_t[:], hT8[:])
        ehT8_t = const.tile([P, KD, B], _F8)
        nc.sync.dma_start(ehT8_t[:], ehT8[:])
        nc.sync.dma_start(tT_t[:], tT[:])

        vc_t = const.tile([P, KC, HSL], _BF16)
        nc.sync.dma_start(vc_t[:], vc[:])
        ub_t = const.tile([P, NCOL], _F32)
        nc.sync.dma_start(ub_t[:], ub[:, :])
        br_t = const.tile([P, 1], _F32)
        nc.sync.dma_start(br_t[:], br[:, :])

        bil_t = const.tile([P, BT, NCOL], _F32)  # bilinear + corr col, b-major
        linb_t = const.tile([P, BT, HSL], _F32)  # linear + bias, b-major
        scores_t = const.tile([P, BT], _F32)

        lsp = ctx.enter_context(tc.tile_pool(name="lst", bufs=2))

        def lin_phase():
            # col-tiled: 4 batch-512 tiles accumulate concurrently in the
            # four 32-partition column groups of one PSUM bank. pl[32j+c, n]
            # = lin^T[h=c, b=(rnd*4+j)*512+n].
            for rnd in range(2):
                pl = psp.tile([P, 512], _F32, name="ps")
                for kc in range(KC):
                    for j in range(4):
                        b512 = rnd * 4 + j
                        if kc < KD:
                            rhs = hT_t[:, kc, b512 * 512:(b512 + 1) * 512]
                        else:
                            rhs = tT_t[:, kc - KD, b512 * 512:(b512 + 1) * 512]
                        nc.tensor.matmul(
                            pl[32 * j:32 * (j + 1), :], vc_t[:, kc, :], rhs,
                            start=(kc == 0), stop=(kc == KC - 1),
                            tile_position=(0, 32 * j),
                        )
                nc.scalar.activation(
                    pl[:], pl[:], mybir.ActivationFunctionType.Identity,
                    bias=br_t[:],
                )
                lin_stage = lsp.tile([P, 512], _F32, name="lst")
                nc.vector.transpose(lin_stage[:], pl[:])
                for j in range(4):
                    blk = lin_stage[32 * j:32 * (j + 1), :].rearrange(
                        "p (q m c) -> p q m c", q=4, m=4
                    )
                    for m in range(4):
                        dst = linb_t[32 * m:32 * (m + 1),
                                     (rnd * 4 + j) * 4:(rnd * 4 + j) * 4 + 4, :]
                        nc.sync.dma_start(dst, blk[:, :, m, :])

        s2p = ctx.enter_context(tc.tile_pool(name="s2", bufs=2))

        def _udot(bt):
            # scores_part[b] = (relu(bil + lin) ++ corr)[b, :] @ (u ++ 0.5)
            s2_t = s2p.tile([P, NCOL], _F32, name="s2")
            nc.vector.scalar_tensor_tensor(
                out=s2_t[:],
                in0=bil_t[:, bt, :],
                scalar=1.0,
                in1=ub_t[:],
                op0=mybir.AluOpType.mult,
                op1=mybir.AluOpType.mult,
                accum_out=scores_t[:, bt:bt + 1],
            )

        def _ttr(ps_t, bt, col):
            s_t = scr.tile([P, D], _F32, name="s")
            nc.vector.scalar_tensor_tensor(
                out=s_t[:],
                in0=ps_t[:],
                scalar=1.0,
                in1=tl_t[:, bt, :],
                op0=mybir.AluOpType.mult,
                op1=mybir.AluOpType.mult,
                accum_out=bil_t[:, bt, col:col + 1],
            )

        def quad(h, bt, w_t):
            ps_t = psp.tile([P, D], _F32, name="ps")
            if is_f8(h):
                for c in range(2):
                    nc.tensor.matmul(
                        ps_t[:],
                        hT8_t[:, 2 * c:2 * c + 2, bt * P:(bt + 1) * P],
                        w_t[:, 2 * c:2 * c + 2, :],
                        start=(c == 0),
                        stop=(c == 1),
                        perf_mode=mybir.MatmulPerfMode.DoubleRow,
                    )
            else:
                for k in range(KD):
                    nc.tensor.matmul(
                        ps_t[:],
                        hT_t[:, k, bt * P:(bt + 1) * P],
                        w_t[:, k, :],
                        start=(k == 0),
                        stop=(k == KD - 1),
                    )
            _ttr(ps_t, bt, h)

        def corr_quad(bt):
            # bil[:, 32] = head@EWu@tail + eh8@W8u8@tail  (= sum_h u_h * fp8
            # quantization error over the DR h's, udot weight +0.5)
            ps_t = psp.tile([P, D], _F32, name="ps")
            for k in range(KD):
                nc.tensor.matmul(
                    ps_t[:],
                    hT_t[:, k, bt * P:(bt + 1) * P],
                    ewu_t[:, k, :],
                    start=(k == 0), stop=False,
                )
            for c in range(2):
                nc.tensor.matmul(
                    ps_t[:],
                    ehT8_t[:, 2 * c:2 * c + 2, bt * P:(bt + 1) * P],
                    w8u8_t[:, 2 * c:2 * c + 2, :],
                    start=False, stop=(c == 1),
                    perf_mode=mybir.MatmulPerfMode.DoubleRow,
                )
            _ttr(ps_t, bt, HSL)

        # --- main sequence ---
        # first two bf16 h's interleaved per bt while streams land
        for bt in range(BT):
            quad(first2[0], bt, w_tiles[first2[0]])
            quad(first2[1], bt, w_tiles[first2[1]])
        w_tiles.pop(first2[0])
        w_tiles.pop(first2[1])

        # correction column (needs hT fully landed + ewu/w8u8/ehT8)
        for bt in range(BT):
            corr_quad(bt)

        for ridx, rnd in enumerate(rounds):
            # prefetch next round's weights (round ~56us, DMA ~2-8us)
            if ridx + 1 < len(rounds):
                for h in rounds[ridx + 1]:
                    load_w(h)
            else:
                for h in last2:
                    load_w(h)
            tiles = [w_tiles.pop(h) for h in rnd]
            for bt in range(BT):
                for h, w_t in zip(rnd, tiles):
                    quad(h, bt, w_t)
            if ridx == LIN_AT - 1:
                lin_phase()

        # last two h's merged bt-major with phase 3
        h30, h31 = last2
        w30 = w_tiles.pop(h30)
        w31 = w_tiles.pop(h31)
        for bt in range(BT):
            quad(h30, bt, w30)
            quad(h31, bt, w31)
            # in-place: bil := relu(bil + lin) on the 32 real columns
            nc.gpsimd.tensor_tensor(
                out=bil_t[:, bt, 0:HSL], in0=bil_t[:, bt, 0:HSL],
                in1=linb_t[:, bt, :], op=mybir.AluOpType.add,
            )
            if bt == BT - 1:
                nc.vector.tensor_scalar_max(
                    bil_t[:, bt, 0:HSL], bil_t[:, bt, 0:HSL], 0.0
                )
            else:
                nc.scalar.activation(
                    bil_t[:, bt, 0:HSL], bil_t[:, bt, 0:HSL],
                    mybir.ActivationFunctionType.Relu,
                )
            if bt >= 1:
                _udot(bt - 1)
            if bt - 1 == 15:
                nc.sync.dma_start(out[:, 0:16], scores_t[:, 0:16])
            if bt - 1 == 30:
                nc.sync.dma_start(out[:, 16:31], scores_t[:, 16:31])

        _udot(BT - 1)
        nc.sync.dma_start(out[:, 31:BT], scores_t[:, 31:BT])

    nc.compile()
    return nc


def _get_nc():
    global _NC_CACHE
    if _NC_CACHE is None:
        _NC_CACHE = _build_nc()
    return _NC_CACHE


def kernel(head_embeddings, relation_embeddings, tail_embeddings, W_R, V_R, u_R, b_R):
    head = np.asarray(head_embeddings, dtype=np.float32)
    tail = np.asarray(tail_embeddings, dtype=np.float32)
    W = np.asarray(W_R, dtype=np.float32)
    V = np.asarray(V_R, dtype=np.float32)
    u = np.asarray(u_R, dtype=np.float32)
    b = np.asarray(b_R, dtype=np.float32)

    bf = ml_dtypes.bfloat16
    f8 = ml_dtypes.float8_e4m3fn

    # |u|-sorted h assignment: smallest-|u| h's take the fp8 slots
    order = np.argsort(np.abs(u), kind="stable")
    n8 = N8C * NCORES
    dr_h = order[:n8].reshape(NCORES, N8C)
    bf_h = order[n8:].reshape(NCORES, NBF)

    # [D, B] -> [P, KD, B]: partition p holds row k*128+p of the transpose
    def to_pkb(x, dt):
        return np.ascontiguousarray(
            x.T.reshape(KD, P, B).transpose(1, 0, 2).astype(dt))

    h8 = head.astype(f8).astype(np.float32)
    eh = head - h8
    hTa = to_pkb(head, bf)
    hT8a = to_pkb(head, f8)
    ehT8a = to_pkb(eh, f8)
    tTa = to_pkb(tail, bf)
    tla = tail.astype(bf)

    def to_pkd(m, dt):
        # [D, D] -> [P, KD, D]
        return np.ascontiguousarray(
            m.reshape(KD, P, D).transpose(1, 0, 2).astype(dt))

    in_maps = []
    for c in range(NCORES):
        hs = np.concatenate([dr_h[c], bf_h[c]])
        Wd = W[dr_h[c]].astype(np.float64)
        ud = u[dr_h[c]].astype(np.float64)
        W8d = Wd.astype(np.float32).astype(f8).astype(np.float64)
        Wu = np.einsum("h,hij->ij", ud, Wd)
        W8u = np.einsum("h,hij->ij", ud, W8d)
        ewu_a = to_pkd((Wu - W8u).astype(np.float32), bf)
        w8u8_a = to_pkd(W8u.astype(np.float32), f8)

        w8a = np.ascontiguousarray(
            W[dr_h[c]].reshape(N8C, KD, P, D).transpose(0, 2, 1, 3).astype(f8))
        wa = np.ascontiguousarray(
            W[bf_h[c]].reshape(NBF, KD, P, D).transpose(0, 2, 1, 3).astype(bf))
        vca = np.ascontiguousarray(
            V[hs].T.reshape(KC, P, HSL).transpose(1, 0, 2).astype(bf))
        uext = np.concatenate([u[hs], np.float32([0.5])]).astype(np.float32)
        in_maps.append({
            "hT": hTa,
            "hT8": hT8a,
            "ehT8": ehT8a,
            "tT": tTa,
            "tl": tla,
            "w": wa,
            "w8": w8a,
            "ewu": ewu_a,
            "w8u8": w8u8_a,
            "vc": vca,
            "ub": np.ascontiguousarray(np.broadcast_to(uext, (P, NCOL))),
            "br": np.ascontiguousarray(np.tile(b[hs], 4).reshape(P, 1)),
        })

    nc = _get_nc()
    trace = bool(int(os.environ.get("BILINEAR_TRACE", "0")))
    res = bass_utils.run_bass_kernel_spmd(
        nc, in_maps, core_ids=list(range(NCORES)), trace=trace
    )
    global LAST_RESULT
    LAST_RESULT = res
    if trace:
        print(f"HW exec time: {res.exec_time_ns} ns")
        if res.instructions_and_trace:
            print(f"trace: {res.instructions_and_trace[1]}")

    acc = np.zeros(B, dtype=np.float64)
    for c in range(NCORES):
        part = res.results[c]["scores_part"]  # [P, BT]
        acc += part.T.reshape(-1).astype(np.float64)
    return acc.astype(np.float32)


# revision 11
# speedup vs baseline: 1.2199x; 1.0155x over previous
"""Trainium2 Bass kernel for BilinearScoringFunction.

scores[b] = relu( einsum('bi,hij,bj->bh', head, W_R, tail)
                  + concat(head, tail) @ V_R.T + b_R ) @ u_R

B=4096, D=512, H=256. Sharded over 8 NeuronCores along the hidden dim H
(32 hidden units per core); the host sums the 8 partial score vectors.

Precision plan (validated against the reference in fp64 simulation, which
reproduces the measured rel_err to 4 digits):
  - h's are assigned to cores by |u_h| rank: the 168 smallest-|u| h's run
    as fp8-e4m3 DoubleRow matmuls (2x PE rate; 21 per core), the 88
    largest-|u| h's in bf16 (11 per core). Since each h's score-error
    contribution scales with u_h^2, u-sorting buys ~5x more fp8 h's than
    a random assignment at the same error (bottom-n of 256 uniform |u|
    carries only (n/256)^2 of the average u^2 weight).
  - a relu mean-correction: relu(x) = x/2 + |x|/2, and the linear half of
    the fp8 quantization error collapses over h:
      sum_h u_h * (bil_h - bil8_h) ~= head@EWu@tail + eh8@W8u8@tail
    with EWu = sum u_h (W_h - W8_h) (bf16, small entries), eh8 =
    fp8(head - fp8(head)), W8u8 = fp8(sum u_h W8_h). Computed as one
    extra pseudo-h column (4 bf16 + 2 DR matmuls into one PSUM + one
    reduce), weighted +0.5 in the u-dot and excluded from the relu.
    Cuts the fp8 error ~sqrt(2)x. Measured rel_l2 = 1.68e-2 (gate 2e-2).

Engine balance (measured): Vector 693us busy / PE 682us busy over a
~711us span -- both saturated. The per-(h,bt) reduce is the hard floor:
every fused 2-stream DVE op (custom TTR, native TTR, STT) runs at 1
elem/cycle/partition regardless of dtype on this RTL (2x_1p only
engages for 1-stream ops), so 32 h x 32 bt x 512 cols ~= 600ns each.

Per core structure:
  per h: T_h = head @ W_h on TensorE (4 accumulating K=128 bf16 matmuls
  or 2 K=256 fp8-DR matmuls, N=512), then one native VectorE
  scalar_tensor_tensor computes bil[:, h] = rowsum(T_h * tail) straight
  out of PSUM (accum_out). h's processed in (DR,DR,BF) rounds
  interleaved per batch tile so the 256-col DR weight loads (no FWL,
  ~213ns each) hide under bf16 matmul streams. Linear term via h-major
  V@concat^T matmuls + 32x32 DVE transposes, bias added on ScalarE.
  Phase 3 (add lin on GpSimd, relu on ScalarE, u-dot over 33 columns on
  VectorE) interleaved with the last two h's, bt-major.
"""

import os
from contextlib import ExitStack

import numpy as np
import ml_dtypes

import concourse.bacc as bacc
import concourse.tile as tile
import concourse.mybir as mybir
from concourse import bass_utils

B, D, H = 4096, 512, 256
NCORES = 8
HSL = H // NCORES          # hidden units per core = 32
N8C = 22                   # fp8-DR h's per core (slots 0..N8C-1)
OFF_SLOTS = (2, 6, 10)     # DR slots whose reduce runs on Scalar+GpSimd
NBF = HSL - N8C            # bf16 h's per core (slots N8C..31)
NCOL = HSL + 1             # bil columns incl. the correction column
P = 128                    # partitions
BT = B // P                # batch tiles of 128 = 32
KD = D // P                # contraction chunks per operand = 4
KC = 2 * KD                # concat contraction chunks = 8
LIN_AT = 3                 # insert linear-term matmuls after this many
                           # h-loop rounds of the main sequence

_F32 = mybir.dt.float32
_BF16 = mybir.dt.bfloat16
_F8 = mybir.dt.float8e4

_NC_CACHE = None


def _build_nc():
    nc = bacc.Bacc(
        "TRN2",
        target_bir_lowering=False,
        debug=False,
        enable_asserts=False,
        num_devices=NCORES,
    )
    # all pre-arranged host-side so every DMA is a clean 2D/3D copy
    hT = nc.dram_tensor("hT", [P, KD, B], _BF16, kind="ExternalInput").ap()
    hT8 = nc.dram_tensor("hT8", [P, KD, B], _F8, kind="ExternalInput").ap()
    ehT8 = nc.dram_tensor("ehT8", [P, KD, B], _F8, kind="ExternalInput").ap()
    tT = nc.dram_tensor("tT", [P, KD, B], _BF16, kind="ExternalInput").ap()
    tl = nc.dram_tensor("tl", [B, D], _BF16, kind="ExternalInput").ap()
    w = nc.dram_tensor("w", [NBF, P, KD, D], _BF16, kind="ExternalInput").ap()
    w8 = nc.dram_tensor("w8", [N8C, P, KD, D], _F8, kind="ExternalInput").ap()
    ewu = nc.dram_tensor("ewu", [P, KD, D], _BF16, kind="ExternalInput").ap()
    w8u8 = nc.dram_tensor("w8u8", [P, KD, D], _F8, kind="ExternalInput").ap()
    vc = nc.dram_tensor("vc", [P, KC, HSL], _BF16, kind="ExternalInput").ap()
    ub = nc.dram_tensor("ub", [P, NCOL], _F32, kind="ExternalInput").ap()
    br = nc.dram_tensor("br", [P, 1], _F32, kind="ExternalInput").ap()
    out = nc.dram_tensor("scores_part", [P, BT], _F32, kind="ExternalOutput").ap()

    with tile.TileContext(nc) as tc, ExitStack() as ctx:
        const = ctx.enter_context(tc.tile_pool(name="const", bufs=1))
        wp = ctx.enter_context(tc.tile_pool(name="w", bufs=4))
        w8p = ctx.enter_context(tc.tile_pool(name="w8", bufs=6))
        psp = ctx.enter_context(tc.tile_pool(name="ps", bufs=8, space="PSUM"))
        scr = ctx.enter_context(tc.tile_pool(name="scr", bufs=2))
        tcp = ctx.enter_context(tc.tile_pool(name="tc8", bufs=3))
        prp = ctx.enter_context(tc.tile_pool(name="prod", bufs=3))
        dpp = ctx.enter_context(tc.tile_pool(name="dump", bufs=2))

        # processing order of per-core h slots: two bf16 h's first (they
        # pace the hT/tl stream landing), then rounds interleaving DR and
        # bf16 h's per batch tile so the 256-col DR weight loads (no FWL,
        # ~213ns each) hide under bf16 matmul streams; the last two bf16
        # slots are merged with phase 3.
        first2 = [N8C, N8C + 1]
        last2 = [N8C - 1, HSL - 1]
        drs = list(range(N8C - 1))
        bfs = list(range(N8C + 2, HSL - 1))
        rounds = []
        di = 0
        for bslot in bfs:
            rounds.append((drs[di], drs[di + 1], bslot))
            di += 2
        while di < len(drs):
            if len(drs) - di == 1:
                rounds[-1] = rounds[-1] + (drs[di],)
                di += 1
            else:
                rounds.append(tuple(drs[di:di + 2]))
                di += 2

        def is_f8(h):
            return h < N8C

        w_tiles = {}

        def load_w(h):
            if is_f8(h):
                t = w8p.tile([P, KD, D], _F8, name="w8t")
                nc.sync.dma_start(t[:], w8[h])
            else:
                t = wp.tile([P, KD, D], _BF16, name="wt")
                nc.sync.dma_start(t[:], w[h - N8C])
            w_tiles[h] = t
            return t

        # --- DMAs in priority order: compute start gates on w[first2[0]] + hT.
        w_tiles[first2[0]] = wp.tile([P, KD, D], _BF16, name="wt")
        nc.sync.dma_start(w_tiles[first2[0]][:, 0:2, :], w[first2[0] - N8C][:, 0:2, :])
        nc.sync.dma_start(w_tiles[first2[0]][:, 2:4, :], w[first2[0] - N8C][:, 2:4, :])

        # head^T / tail as interleaved 2-batch-tile chunks: the first-h matmul
        # quad for tile bt only gates on its 256KB hT chunk, so compute
        # starts early and the first two h's run DMA-paced as streams land.
        hT_t = const.tile([P, KD, B], _BF16)
        tT_t = const.tile([P, KD, B], _BF16)
        tl_t = const.tile([P, BT, D], _BF16)
        nc.scalar.dma_start(hT_t[:, 0:2, 0:P], hT[:, 0:2, 0:P])
        nc.scalar.dma_start(hT_t[:, 2:4, 0:P], hT[:, 2:4, 0:P])
        nc.scalar.dma_start(tl_t[:, 0:1, :], tl[0:P, :].rearrange(
            "(t p) d -> p t d", p=P))
        load_w(first2[1])
        nc.sync.dma_start(hT_t[:, :, P:2 * P], hT[:, :, P:2 * P])
        nc.sync.dma_start(tl_t[:, 1:2, :], tl[P:2 * P, :].rearrange(
            "(t p) d -> p t d", p=P))
        for bt in range(2, BT, 2):
            sl = slice(bt * P, (bt + 2) * P)
            nc.sync.dma_start(hT_t[:, :, sl], hT[:, :, sl])
            nc.sync.dma_start(tl_t[:, bt:bt + 2, :], tl[sl, :].rearrange(
                "(t p) d -> p t d", p=P))
            if bt == 8:
                # correction + first mid-seq weights: late enough not to
                # displace the stream chunks, early enough to land before use
                ewu_t = const.tile([P, KD, D], _BF16)
                nc.sync.dma_start(ewu_t[:], ewu[:])
                w8u8_t = const.tile([P, KD, D], _F8)
                nc.sync.dma_start(w8u8_t[:], w8u8[:])
                for h in rounds[0]:
                    load_w(h)
        # bulk tensors needed later: fp8 head copies, tail^T (lin phase)
        hT8_t = const.tile([P, KD, B], _F8)
        nc.sync.dma_start(hT8_t[:], hT8[:])
        ehT8_t = const.tile([P, KD, B], _F8)
        nc.sync.dma_start(ehT8_t[:], ehT8[:])
        nc.sync.dma_start(tT_t[:], tT[:])

        vc_t = const.tile([P, KC, HSL], _BF16)
        nc.sync.dma_start(vc_t[:], vc[:])
        ub_t = const.tile([P, NCOL], _F32)
        nc.sync.dma_start(ub_t[:], ub[:, :])
        br_t = const.tile([P, 1], _F32)
        nc.sync.dma_start(br_t[:], br[:, :])

        bil_t = const.tile([P, BT, NCOL], _F32)  # bilinear + corr col, b-major
        linb_t = const.tile([P, BT, HSL], _F32)  # linear + bias, b-major
        scores_t = const.tile([P, BT], _F32)

        lsp = ctx.enter_context(tc.tile_pool(name="lst", bufs=2))

        def lin_phase():
            # col-tiled: 4 batch-512 tiles accumulate concurrently in the
            # four 32-partition column groups of one PSUM bank. pl[32j+c, n]
            # = lin^T[h=c, b=(rnd*4+j)*512+n].
            for rnd in range(2):
                pl = psp.tile([P, 512], _F32, name="ps")
                for kc in range(KC):
                    for j in range(4):
                        b512 = rnd * 4 + j
                        if kc < KD:
                            rhs = hT_t[:, kc, b512 * 512:(b512 + 1) * 512]
                        else:
                            rhs = tT_t[:, kc - KD, b512 * 512:(b512 + 1) * 512]
                        nc.tensor.matmul(
                            pl[32 * j:32 * (j + 1), :], vc_t[:, kc, :], rhs,
                            start=(kc == 0), stop=(kc == KC - 1),
                            tile_position=(0, 32 * j),
                        )
                nc.scalar.activation(
                    pl[:], pl[:], mybir.ActivationFunctionType.Identity,
                    bias=br_t[:],
                )
                lin_stage = lsp.tile([P, 512], _F32, name="lst")
                nc.vector.transpose(lin_stage[:], pl[:])
                for j in range(4):
                    blk = lin_stage[32 * j:32 * (j + 1), :].rearrange(
                        "p (q m c) -> p q m c", q=4, m=4
                    )
                    for m in range(4):
                        dst = linb_t[32 * m:32 * (m + 1),
                                     (rnd * 4 + j) * 4:(rnd * 4 + j) * 4 + 4, :]
                        nc.sync.dma_start(dst, blk[:, :, m, :])

        s2p = ctx.enter_context(tc.tile_pool(name="s2", bufs=2))

        def _udot(bt):
            # scores_part[b] = (relu(bil + lin) ++ corr)[b, :] @ (u ++ 0.5)
            s2_t = s2p.tile([P, NCOL], _F32, name="s2")
            nc.vector.scalar_tensor_tensor(
                out=s2_t[:],
                in0=bil_t[:, bt, :],
                scalar=1.0,
                in1=ub_t[:],
                op0=mybir.AluOpType.mult,
                op1=mybir.AluOpType.mult,
                accum_out=scores_t[:, bt:bt + 1],
            )

        def _ttr(ps_t, bt, col):
            s_t = scr.tile([P, D], _F32, name="s")
            nc.vector.scalar_tensor_tensor(
                out=s_t[:],
                in0=ps_t[:],
                scalar=1.0,
                in1=tl_t[:, bt, :],
                op0=mybir.AluOpType.mult,
                op1=mybir.AluOpType.mult,
                accum_out=bil_t[:, bt, col:col + 1],
            )

        def _off_reduce(ps_t, bt, col):
            # Vector-free reduce: ScalarE casts PSUM->bf16, GpSimd multiplies
            # by tail, ScalarE row-sums via the activation accumulator.
            tcast = tcp.tile([P, D], _BF16, name="tc")
            nc.scalar.activation(
                tcast[:], ps_t[:], mybir.ActivationFunctionType.Copy)
            prod = prp.tile([P, D], _BF16, name="pr")
            nc.gpsimd.tensor_tensor(
                out=prod[:], in0=tcast[:], in1=tl_t[:, bt, :],
                op=mybir.AluOpType.mult)
            dump = dpp.tile([P, D], _BF16, name="dp")
            nc.scalar.activation(
                dump[:], prod[:], mybir.ActivationFunctionType.Copy,
                accum_out=bil_t[:, bt, col:col + 1])

        def quad(h, bt, w_t):
            ps_t = psp.tile([P, D], _F32, name="ps")
            if is_f8(h):
                for c in range(2):
                    nc.tensor.matmul(
                        ps_t[:],
                        hT8_t[:, 2 * c:2 * c + 2, bt * P:(bt + 1) * P],
                        w_t[:, 2 * c:2 * c + 2, :],
                        start=(c == 0),
                        stop=(c == 1),
                        perf_mode=mybir.MatmulPerfMode.DoubleRow,
                    )
            else:
                for k in range(KD):
                    nc.tensor.matmul(
                        ps_t[:],
                        hT_t[:, k, bt * P:(bt + 1) * P],
                        w_t[:, k, :],
                        start=(k == 0),
                        stop=(k == KD - 1),
                    )
            if h in OFF_SLOTS:
                _off_reduce(ps_t, bt, h)
            else:
                _ttr(ps_t, bt, h)

        def corr_quad(bt):
            # bil[:, 32] = head@EWu@tail + eh8@W8u8@tail  (= sum_h u_h * fp8
            # quantization error over the DR h's, udot weight +0.5)
            ps_t = psp.tile([P, D], _F32, name="ps")
            for k in range(KD):
                nc.tensor.matmul(
                    ps_t[:],
                    hT_t[:, k, bt * P:(bt + 1) * P],
                    ewu_t[:, k, :],
                    start=(k == 0), stop=False,
                )
            for c in range(2):
                nc.tensor.matmul(
                    ps_t[:],
                    ehT8_t[:, 2 * c:2 * c + 2, bt * P:(bt + 1) * P],
                    w8u8_t[:, 2 * c:2 * c + 2, :],
                    start=False, stop=(c == 1),
                    perf_mode=mybir.MatmulPerfMode.DoubleRow,
                )
            _ttr(ps_t, bt, HSL)

        # --- main sequence ---
        # first two bf16 h's interleaved per bt while streams land
        for bt in range(BT):
            quad(first2[0], bt, w_tiles[first2[0]])
            quad(first2[1], bt, w_tiles[first2[1]])
        w_tiles.pop(first2[0])
        w_tiles.pop(first2[1])

        # correction column (needs hT fully landed + ewu/w8u8/ehT8)
        for bt in range(BT):
            corr_quad(bt)

        for ridx, rnd in enumerate(rounds):
            # prefetch next round's weights (round ~56us, DMA ~2-8us)
            if ridx + 1 < len(rounds):
                for h in rounds[ridx + 1]:
                    load_w(h)
            else:
                for h in last2:
                    load_w(h)
            tiles = [w_tiles.pop(h) for h in rnd]
            for bt in range(BT):
                for h, w_t in zip(rnd, tiles):
                    quad(h, bt, w_t)
            if ridx == LIN_AT - 1:
                lin_phase()

        # last two h's merged bt-major with phase 3
        h30, h31 = last2
        w30 = w_tiles.pop(h30)
        w31 = w_tiles.pop(h31)
        for bt in range(BT):
            quad(h30, bt, w30)
            quad(h31, bt, w31)
            # in-place: bil := relu(bil + lin) on the 32 real columns
            nc.gpsimd.tensor_tensor(
                out=bil_t[:, bt, 0:HSL], in0=bil_t[:, bt, 0:HSL],
                in1=linb_t[:, bt, :], op=mybir.AluOpType.add,
            )
            if bt == BT - 1:
                nc.vector.tensor_scalar_max(
                    bil_t[:, bt, 0:HSL], bil_t[:, bt, 0:HSL], 0.0
                )
            else:
                nc.scalar.activation(
                    bil_t[:, bt, 0:HSL], bil_t[:, bt, 0:HSL],
                    mybir.ActivationFunctionType.Relu,
                )
            if bt >= 1:
                _udot(bt - 1)
            if bt - 1 == 15:
                nc.sync.dma_start(out[:, 0:16], scores_t[:, 0:16])
            if bt - 1 == 30:
                nc.sync.dma_start(out[:, 16:31], scores_t[:, 16:31])

        _udot(BT - 1)
        nc.sync.dma_start(out[:, 31:BT], scores_t[:, 31:BT])

    nc.compile()
    return nc


def _get_nc():
    global _NC_CACHE
    if _NC_CACHE is None:
        _NC_CACHE = _build_nc()
    return _NC_CACHE


def kernel(head_embeddings, relation_embeddings, tail_embeddings, W_R, V_R, u_R, b_R):
    head = np.asarray(head_embeddings, dtype=np.float32)
    tail = np.asarray(tail_embeddings, dtype=np.float32)
    W = np.asarray(W_R, dtype=np.float32)
    V = np.asarray(V_R, dtype=np.float32)
    u = np.asarray(u_R, dtype=np.float32)
    b = np.asarray(b_R, dtype=np.float32)

    bf = ml_dtypes.bfloat16
    f8 = ml_dtypes.float8_e4m3fn

    # |u|-sorted h assignment: smallest-|u| h's take the fp8 slots
    order = np.argsort(np.abs(u), kind="stable")
    n8 = N8C * NCORES
    dr_h = order[:n8].reshape(NCORES, N8C)
    bf_h = order[n8:].reshape(NCORES, NBF)

    # [D, B] -> [P, KD, B]: partition p holds row k*128+p of the transpose
    def to_pkb(x, dt):
        return np.ascontiguousarray(
            x.T.reshape(KD, P, B).transpose(1, 0, 2).astype(dt))

    h8 = head.astype(f8).astype(np.float32)
    eh = head - h8
    hTa = to_pkb(head, bf)
    hT8a = to_pkb(head, f8)
    ehT8a = to_pkb(eh, f8)
    tTa = to_pkb(tail, bf)
    tla = tail.astype(bf)

    def to_pkd(m, dt):
        # [D, D] -> [P, KD, D]
        return np.ascontiguousarray(
            m.reshape(KD, P, D).transpose(1, 0, 2).astype(dt))

    in_maps = []
    for c in range(NCORES):
        hs = np.concatenate([dr_h[c], bf_h[c]])
        Wd = W[dr_h[c]].astype(np.float64)
        ud = u[dr_h[c]].astype(np.float64)
        W8d = Wd.astype(np.float32).astype(f8).astype(np.float64)
        Wu = np.einsum("h,hij->ij", ud, Wd)
        W8u = np.einsum("h,hij->ij", ud, W8d)
        ewu_a = to_pkd((Wu - W8u).astype(np.float32), bf)
        w8u8_a = to_pkd(W8u.astype(np.float32), f8)

        w8a = np.ascontiguousarray(
            W[dr_h[c]].reshape(N8C, KD, P, D).transpose(0, 2, 1, 3).astype(f8))
        wa = np.ascontiguousarray(
            W[bf_h[c]].reshape(NBF, KD, P, D).transpose(0, 2, 1, 3).astype(bf))
        vca = np.ascontiguousarray(
            V[hs].T.reshape(KC, P, HSL).transpose(1, 0, 2).astype(bf))
        uext = np.concatenate([u[hs], np.float32([0.5])]).astype(np.float32)
        in_maps.append({
            "hT": hTa,
            "hT8": hT8a,
            "ehT8": ehT8a,
            "tT": tTa,
            "tl": tla,
            "w": wa,
            "w8": w8a,
            "ewu": ewu_a,
            "w8u8": w8u8_a,
            "vc": vca,
            "ub": np.ascontiguousarray(np.broadcast_to(uext, (P, NCOL))),
            "br": np.ascontiguousarray(np.tile(b[hs], 4).reshape(P, 1)),
        })

    nc = _get_nc()
    trace = bool(int(os.environ.get("BILINEAR_TRACE", "0")))
    res = bass_utils.run_bass_kernel_spmd(
        nc, in_maps, core_ids=list(range(NCORES)), trace=trace
    )
    global LAST_RESULT
    LAST_RESULT = res
    if trace:
        print(f"HW exec time: {res.exec_time_ns} ns")
        if res.instructions_and_trace:
            print(f"trace: {res.instructions_and_trace[1]}")

    acc = np.zeros(B, dtype=np.float64)
    for c in range(NCORES):
        part = res.results[c]["scores_part"]  # [P, BT]
        acc += part.T.reshape(-1).astype(np.float64)
    return acc.astype(np.float32)
